# revision 1
# baseline (speedup 1.0000x reference)
"""Trainium2 Bass kernel for nn_DataAugmentation (flip + resized-crop +
brightness/contrast/saturation/hue) — 8-core data-parallel.

Self-contained: takes FULL inputs, shards batch across 8 NeuronCores,
runs one Bass/Tile program per core via run_bass_kernel_spmd, gathers.
"""

import numpy as np

import concourse.bass as bass
import concourse.bacc as bacc
import concourse.tile as tile
import concourse.mybir as mybir
from concourse.bass_utils import run_bass_kernel_spmd
from concourse.dve_spec import (
    Spec, Src0, Src1, C0, C1, C2, Zero, One, maxx, minn, select, Bin, AluOp,
    lower,
)
from concourse import dve_ops as _dops
from concourse.dve_ops import DveOp, DveOpSpec, OPS, CUSTOM_DVE_SPECS, _SUB_OPCODE_FOR_NAME, has_src1

F32 = mybir.dt.float32
P = 128
OUT = 64
N_CORES = 8
B_FULL = 4096
B_CORE = B_FULL // N_CORES          # 512
GPAIRS = 16                         # pairs per group
NPAIR = B_CORE // 2                 # 256
NGROUP = NPAIR // GPAIRS            # 16
NFAC = 6                            # bf, cf, sf, osf, cb, hf6
GRAY_W = (0.2989, 0.587, 0.114)


# ---------------------------------------------------------------- custom ops
def _register_op(name, spec):
    if name in _SUB_OPCODE_FOR_NAME:
        for o in OPS:
            if o.name == name:
                return o
    opc = 1 + len(OPS)
    _SUB_OPCODE_FOR_NAME[name] = opc
    shas = {}
    for ver in ("v3", "v4"):
        try:
            s = DveOpSpec(name=name, opcode=opc, uops=lower(spec, ver=ver),
                          rd1_en=has_src1(spec))
            shas[ver] = s.sha(ver)
        except ValueError:
            pass
    op = DveOp(name, spec, subdim=False, uops_sha=shas)
    OPS.append(op)
    CUSTOM_DVE_SPECS[name] = spec
    return op


def _refbc(v, like):
    """Broadcast a [P,1] per-partition scalar (or python float) over `like`."""
    if isinstance(v, np.ndarray) and v.ndim >= 1:
        return v.reshape(v.shape[0], *([1] * (like.ndim - 1))).astype(np.float32)
    return np.float32(v)


def _refsame(v, like):
    """Reshape/broadcast an in1 operand to in0's shape."""
    if v.shape == like.shape:
        return v
    if v.size == like.size:
        return v.reshape(like.shape)
    return np.broadcast_to(v.reshape(v.shape[0], 1, -1) if v.ndim == 2 else v, like.shape)


def _absd(a, b):
    return Bin(AluOp.ABSOLUTE_DIFF, a, b)


# hat(x) = relu(1 - |x - c|): bilinear interp row weight
HAT = _register_op("AUG_HAT", Spec(
    body=maxx(One - _absd(Src0, C0), Zero),
    reference=lambda in0, in1, s0, s1, imm2:
        np.maximum(1.0 - np.abs(in0 - _refbc(s0, in0)), 0.0).astype(np.float32),
))
# tri(z) = min(|z-c1|, |z-c2|)
TRI = _register_op("AUG_TRI", Spec(
    body=minn(_absd(Src0, C0), _absd(Src0, C1)),
    reference=lambda in0, in1, s0, s1, imm2:
        np.minimum(np.abs(in0 - _refbc(s0, in0)), np.abs(in0 - _refbc(s1, in0))).astype(np.float32),
))
# qw = cr * clamp01(c0 - tri)
QW = _register_op("AUG_QW", Spec(
    body=Src1 * minn(maxx(Bin(AluOp.SUBTRACT, C0, Src0), Zero), One),
    reference=lambda in0, in1, s0, s1, imm2:
        (_refsame(in1, in0) * np.minimum(np.maximum(_refbc(s0, in0) - in0, 0.0), 1.0)).astype(np.float32),
))
# zb = mr ? 0 : (mg ? c2 : 2*c2)
ZB0 = _register_op("AUG_ZB0", Spec(
    body=select(Src0, Zero, select(Src1, C2, C2 + C2)),
    reference=lambda in0, in1, s0, s1, imm2:
        np.where(in0 != 0, 0.0, np.where(in1 != 0, imm2, 2 * imm2)).astype(np.float32),
))
# g2 = in0*c0 + in1*c1 (grayscale partial)
G2 = _register_op("AUG_G2", Spec(
    body=Src0 * C0 + Src1 * C1,
    reference=lambda in0, in1, s0, s1, imm2:
        (in0 * _refbc(s0, in0) + _refsame(in1, in0) * np.float32(s1)).astype(np.float32),
))
# cre1 = max(|in0|, |in1|); cre2 = max(in0, |in1|) + c0
CRE1 = _register_op("AUG_CRE1", Spec(
    body=maxx(maxx(Src0, Zero - Src0), maxx(Src1, Zero - Src1)),
    reference=lambda in0, in1, s0, s1, imm2:
        np.maximum(np.abs(in0), np.abs(_refsame(in1, in0))).astype(np.float32),
))
CRE2 = _register_op("AUG_CRE2", Spec(
    body=maxx(Src0, maxx(Src1, Zero - Src1)) + C0,
    reference=lambda in0, in1, s0, s1, imm2:
        (np.maximum(in0, np.abs(_refsame(in1, in0))) + np.float32(s0)).astype(np.float32),
))
# satcl = clamp01(in0*c0 + in1)   (in1 may be broadcast-shaped)
SATCL = _register_op("AUG_SATCL", Spec(
    body=minn(maxx(Src0 * C0 + Src1, Zero), One),
    reference=lambda in0, in1, s0, s1, imm2:
        np.clip(in0 * _refbc(s0, in0) + _refsame(in1, in0), 0.0, 1.0).astype(np.float32),
))


# ---------------------------------------------------------------- device program
def build_nc(b_core=B_CORE, gpairs=GPAIRS, debug=False):
    npair = b_core // 2
    ngroup = npair // gpairs
    assert ngroup * gpairs == npair
    G = gpairs
    FDP = OUT * G          # pixel-class free size per group
    AluT = mybir.AluOpType
    Act = mybir.ActivationFunctionType

    nc = bacc.Bacc("TRN2", target_bir_lowering=False, debug=debug)

    x_in = nc.dram_tensor("x_in", [b_core, 3, OUT, OUT], F32, kind="ExternalInput")
    ysv_in = nc.dram_tensor("ysv", [b_core, OUT], F32, kind="ExternalInput")
    xsv_in = nc.dram_tensor("xsv", [b_core, OUT], F32, kind="ExternalInput")
    fac_in = nc.dram_tensor("fac", [ngroup, P, NFAC * G], F32, kind="ExternalInput")
    iota_in = nc.dram_tensor("iota", [P, 2], F32, kind="ExternalInput")
    o2_in = nc.dram_tensor("o2", [P, P], F32, kind="ExternalInput")
    out_d = nc.dram_tensor("out", [b_core, 3, OUT, OUT], F32, kind="ExternalOutput")

    with tile.TileContext(nc) as tc:
        with tc.tile_pool(name="persist", bufs=1) as pers, \
             tc.tile_pool(name="grp", bufs=2) as grp, \
             tc.tile_pool(name="hue", bufs=2) as hue, \
             tc.tile_pool(name="pp", bufs=2, space="PSUM") as pp:

            IOTA = pers.tile([P, 2], F32)
            O2 = pers.tile([P, P], F32)
            nc.sync.dma_start(IOTA[:], iota_in[:])
            nc.sync.dma_start(O2[:], o2_in[:])

            imgd = [pers.tile([P, 384 * G], F32, tag=f"imgd{i}", name=f"imgd{i}") for i in range(2)]
            ryd = [pers.tile([P, 128 * G], F32, tag=f"ryd{i}", name=f"ryd{i}") for i in range(2)]
            for t in imgd + ryd:
                nc.gpsimd.memset(t[:], 0.0)

            xev = x_in[:].rearrange("(q s) c y n -> q s c y n", s=2)
            oev = out_d[:].rearrange("(q s) c y n -> q s c y n", s=2)
            yv = ysv_in[:].rearrange("(q s) k -> q s k", s=2)
            xv = xsv_in[:].rearrange("(q s) k -> q s k", s=2)

            for g in range(ngroup):
                eo = g & 1
                p0 = g * G
                img = imgd[eo]
                ry = ryd[eo]
                imgr = img[:].rearrange("p (gg c k) -> p gg c k", c=3, k=128)
                ryr = ry[:].rearrange("p (gg k) -> p gg k", k=128)

                # ---- input DMAs
                for s in range(2):   # A-half rows 0:64 / B-half 64:128
                    r0, r1 = (0, 64) if s == 0 else (64, 128)
                    c0, c1 = (0, 64) if s == 0 else (64, 128)
                    for c in range(3):
                        nc.sync.dma_start(
                            imgr[r0:r1, :, c, c0:c1],
                            xev[p0:p0 + G, s, c].transpose([1, 0, 2]))
                    nc.sync.dma_start(
                        ryr[r0:r1, :, c0:c1],
                        yv[p0:p0 + G, s, :].unsqueeze(0).broadcast_to((64, G, OUT)))

                rx = grp.tile([P, OUT * G], F32, tag="rx")
                rxr = rx[:].rearrange("p (gg k) -> p gg k", k=OUT)
                for s in range(2):
                    r0, r1 = (0, 64) if s == 0 else (64, 128)
                    nc.sync.dma_start(
                        rxr[r0:r1, :, :],
                        xv[p0:p0 + G, s, :].unsqueeze(0).broadcast_to((64, G, OUT)))

                FAC = grp.tile([P, NFAC * G], F32, tag="fac")
                nc.sync.dma_start(FAC[:], fac_in[g])
                facr = FAC[:].rearrange("p (gg s) -> p gg s", s=NFAC)

                # ---- build interp weight matrices in place (hat of |k - pos|)
                # positions and iota are shifted by +2 so hat(0 - k) == 0 on the
                # zero off-diag blocks; run full-partition on ACT (partition-
                # offset custom ops are a silent no-op on HW; ACT has slack).
                for t_ap in (ry[:], rx[:]):
                    nc.scalar.activation(t_ap, t_ap, Act.Abs, bias=IOTA[:, 1:2])
                    nc.scalar.activation(t_ap, t_ap, Act.Relu, bias=1.0, scale=-1.0)

                # ---- per-pair resize matmuls + brightness
                xbuf = grp.tile([P, 192 * G], F32, tag="xbuf")
                for p in range(G):
                    T1 = pp.tile([P, 384], F32, tag="t1", bufs=3)
                    for c in range(3):
                        nc.tensor.matmul(T1[:, 128 * c:128 * (c + 1)],
                                         imgr[:, p, c, :], ryr[:, p, :],
                                         start=True, stop=True)
                    sbt = grp.tile([P, 384], F32, tag="sbt", bufs=4)
                    nc.scalar.copy(sbt[:], T1[:])
                    T2 = pp.tile([P, 192], F32, tag="t2", bufs=3)
                    for c in range(3):
                        nc.tensor.matmul(T2[:, 64 * c:64 * (c + 1)],
                                         sbt[:, 128 * c:128 * (c + 1)],
                                         rxr[:, p, :], start=True, stop=True)
                    # brightness: x1 = min(x0 * bf, 1)  (PSUM -> SBUF)
                    nc.vector.tensor_scalar(
                        xbuf[:, 192 * p:192 * (p + 1)], T2[:],
                        facr[:, p, 0:1], 1.0, AluT.mult, AluT.min)

                xr = xbuf[:].rearrange("p (gg c k) -> p gg c k", c=3, k=OUT)

                # ---- contrast mean: gray1 of x1, per-sample sums
                gray = grp.tile([P, FDP], F32, tag="gray")
                grayr = gray[:].rearrange("p (gg k) -> p gg k", k=OUT)
                nc.vector._custom_dve(G2, out=grayr[:, :, :], in0=xr[:, :, 0, :],
                                      in1=xr[:, :, 1, :], s0=GRAY_W[0], s1=GRAY_W[1])
                nc.vector.scalar_tensor_tensor(
                    grayr[:, :, :], xr[:, :, 2, :], GRAY_W[2], grayr[:, :, :],
                    AluT.mult, AluT.add)
                mrow = grp.tile([P, G], F32, tag="mrow")
                tri = hue.tile([P, FDP], F32, tag="tri")
                for p in range(G):
                    nc.scalar.activation(tri[:, OUT * p:OUT * (p + 1)],
                                         grayr[:, p, :], Act.Copy,
                                         accum_out=mrow[:, p:p + 1])
                Mcol = pp.tile([P, G], F32, tag="mcol")
                nc.tensor.matmul(Mcol[:], O2[:], mrow[:], start=True, stop=True)
                tb8 = grp.tile([P, G], F32, tag="tb8")
                nc.vector.tensor_tensor(tb8[:], Mcol[:], facr[:, :, 4], AluT.mult)

                # ---- contrast: x2 = clamp01(cf*x1 + tb)  (in place)
                for p in range(G):
                    nc.scalar.activation(xbuf[:, 192 * p:192 * (p + 1)],
                                         xbuf[:, 192 * p:192 * (p + 1)],
                                         Act.Identity, bias=tb8[:, p:p + 1],
                                         scale=facr[:, p, 1:2])
                nc.vector.tensor_scalar(xbuf[:], xbuf[:], 0.0, 1.0, AluT.max, AluT.min)
                x2r = xr

                # ---- saturation: gray2 from clamped x2; x3 = clamp01(sf*x2 + osf*gray2)
                nc.vector._custom_dve(G2, out=grayr[:, :, :], in0=x2r[:, :, 0, :],
                                      in1=x2r[:, :, 1, :], s0=GRAY_W[0], s1=GRAY_W[1])
                nc.vector.scalar_tensor_tensor(
                    grayr[:, :, :], x2r[:, :, 2, :], GRAY_W[2], grayr[:, :, :],
                    AluT.mult, AluT.add)
                g2s = grp.tile([P, FDP], F32, tag="g2s")
                g2sr = g2s[:].rearrange("p (gg k) -> p gg k", k=OUT)
                for p in range(G):
                    nc.scalar.activation(g2sr[:, p, :], grayr[:, p, :], Act.Copy,
                                         scale=facr[:, p, 3:4])
                x3r = xr
                for p in range(G):
                    nc.vector._custom_dve(
                        SATCL, out=x3r[:, p, :, :], in0=x3r[:, p, :, :],
                        in1=g2sr[:, p, :].unsqueeze(1).broadcast_to((P, 3, OUT)),
                        s0=facr[:, p, 2:3])

                # ---- hue
                r_s = x3r[:, :, 0, :]
                g_s = x3r[:, :, 1, :]
                b_s = x3r[:, :, 2, :]
                tA = hue.tile([P, FDP], F32, tag="tA")   # mx1 / mn scratch
                mx = hue.tile([P, FDP], F32, tag="mx")
                cre = hue.tile([P, FDP], F32, tag="cre")
                rcr = hue.tile([P, FDP], F32, tag="rcr")
                esel = hue.tile([P, FDP], F32, tag="esel")
                e1 = hue.tile([P, FDP], F32, tag="e1")
                e2 = hue.tile([P, FDP], F32, tag="e2")
                mr = hue.tile([P, FDP], mybir.dt.uint8, tag="mr")
                mg = hue.tile([P, FDP], mybir.dt.uint8, tag="mg")
                tAr = tA[:].rearrange("p (gg k) -> p gg k", k=OUT)
                mxr = mx[:].rearrange("p (gg k) -> p gg k", k=OUT)

                eselr = esel[:].rearrange("p (gg k) -> p gg k", k=OUT)
                e1r = e1[:].rearrange("p (gg k) -> p gg k", k=OUT)
                e2r = e2[:].rearrange("p (gg k) -> p gg k", k=OUT)
                nc.vector.tensor_tensor(eselr[:, :, :], r_s, g_s, AluT.subtract)  # e3
                nc.vector.tensor_tensor(e1r[:, :, :], g_s, b_s, AluT.subtract)
                nc.vector.tensor_tensor(e2r[:, :, :], b_s, r_s, AluT.subtract)
                # cre = max(|e1|,|e2|,|e3|) + eps  (== mx - mn + eps)
                nc.vector._custom_dve(CRE1, out=cre[:], in0=e1[:], in1=e2[:])
                nc.vector._custom_dve(CRE2, out=cre[:], in0=cre[:], in1=esel[:], s0=1e-20)
                nc.vector.reciprocal_approx_fast(rcr[:], cre[:])
                nc.vector.tensor_tensor(tA[:], r_s, g_s, AluT.max)
                nc.vector.tensor_tensor(mxr[:, :, :], tAr[:, :, :], b_s, AluT.max)
                nc.vector.tensor_tensor(mr[:].rearrange("p (gg k) -> p gg k", k=OUT),
                                        mxr[:, :, :], r_s, AluT.is_equal)
                nc.vector.tensor_tensor(mg[:].rearrange("p (gg k) -> p gg k", k=OUT),
                                        mxr[:, :, :], g_s, AluT.is_equal)
                nc.vector.copy_predicated(esel[:], mg[:], e2[:])
                nc.vector.copy_predicated(esel[:], mr[:], e1[:])
                # zb -> e2 tile (reuse); h6 -> e1 tile (reuse); z -> esel
                nc.vector._custom_dve(ZB0, out=e2[:], in0=mr[:], in1=mg[:], imm2=2.0)
                nc.vector.tensor_tensor(e1[:], esel[:], rcr[:], AluT.mult)
                nc.vector.tensor_tensor(esel[:], e1[:], e2[:], AluT.add)
                zt = esel
                ztr = zt[:].rearrange("p (gg k) -> p gg k", k=OUT)
                nc.vector.tensor_tensor(
                    ztr[:, :, :], ztr[:, :, :],
                    facr[:, :, 5].unsqueeze(2).broadcast_to((P, G, OUT)), AluT.add)
                # out_n = mx - cr * trap(z), n = 5(r), 3(g), 1(b)
                for ci, (cc1, cc2) in enumerate(((-3.0, 3.0), (-1.0, 5.0), (1.0, 7.0))):
                    nc.vector._custom_dve(TRI, out=tri[:], in0=zt[:], s0=cc1, s1=cc2)
                    nc.vector._custom_dve(QW, out=tri[:], in0=tri[:], in1=cre[:], s0=2.0)
                    nc.vector.tensor_tensor(x3r[:, :, ci, :], mxr[:, :, :],
                                            tri[:].rearrange("p (gg k) -> p gg k", k=OUT),
                                            AluT.subtract)

                # ---- output DMAs
                for s in range(2):
                    r0, r1 = (0, 64) if s == 0 else (64, 128)
                    for c in range(3):
                        nc.sync.dma_start(
                            oev[p0:p0 + G, s, c].transpose([1, 0, 2]),
                            x3r[r0:r1, :, c, :])

    nc.compile()
    return nc


# ---------------------------------------------------------------- host prep
def host_prep(x, flip_mask, crop_i, crop_j, crop_h, crop_w,
              b_factor, c_factor, s_factor, h_factor,
              b_core=B_CORE, gpairs=GPAIRS):
    f32 = np.float32
    B = x.shape[0]
    npair = b_core // 2
    ngroup = npair // gpairs
    G = gpairs

    ar = (np.arange(OUT, dtype=f32) + f32(0.5))
    ys = crop_i[:, None].astype(f32) + ar[None, :] * (crop_h.astype(f32)[:, None] / f32(OUT)) - f32(0.5)
    xs = crop_j[:, None].astype(f32) + ar[None, :] * (crop_w.astype(f32)[:, None] / f32(OUT)) - f32(0.5)

    def eff(p):
        return np.where(p < 0, p + f32(1.0), np.minimum(p, f32(63.0))).astype(f32)

    ysv = (eff(ys) + f32(2.0)).astype(f32)
    xsv = (np.where(flip_mask[:, None], f32(63.0) - eff(xs), eff(xs)) + f32(2.0)).astype(f32)

    bf = b_factor.astype(f32)
    cf = c_factor.astype(f32)
    sf = s_factor.astype(f32)
    osf = (f32(1.0) - sf).astype(f32)
    cb = ((f32(1.0) - cf) / f32(OUT * OUT * 1.0)).astype(f32) / f32(1.0)
    cb = ((f32(1.0) - cf) / f32(4096.0)).astype(f32)
    hf6 = (f32(6.0) * h_factor.astype(f32)).astype(f32)

    kk = np.concatenate([np.arange(64, dtype=f32)] * 2) + f32(2.0)
    iota = np.stack([kk, -kk], axis=1).astype(f32)
    o2 = np.zeros((P, P), dtype=f32)
    o2[:64, :64] = 1.0
    o2[64:, 64:] = 1.0

    per_core = []
    n_cores = B // b_core
    for k in range(n_cores):
        sl = slice(k * b_core, (k + 1) * b_core)
        fac = np.zeros((ngroup, P, NFAC * G), dtype=f32)
        vals = np.stack([bf[sl], cf[sl], sf[sl], osf[sl], cb[sl], hf6[sl]], -1)  # [b_core, 6]
        vals = vals.reshape(ngroup, G, 2, NFAC)
        for s, rows in ((0, slice(0, 64)), (1, slice(64, 128))):
            v = vals[:, :, s, :].reshape(ngroup, 1, G * NFAC)
            fac[:, rows, :] = np.broadcast_to(v, (ngroup, 64, G * NFAC))
        per_core.append({
            "x_in": np.ascontiguousarray(x[sl].astype(f32)),
            "ysv": np.ascontiguousarray(ysv[sl]),
            "xsv": np.ascontiguousarray(xsv[sl]),
            "fac": np.ascontiguousarray(fac),
            "iota": iota,
            "o2": o2,
        })
    return per_core


_NC_CACHE = {}


def kernel(**inputs):
    x = np.asarray(inputs["x"], dtype=np.float32)
    args = {k: np.asarray(inputs[k]) for k in
            ("flip_mask", "crop_i", "crop_j", "crop_h", "crop_w",
             "b_factor", "c_factor", "s_factor", "h_factor")}
    in_maps = host_prep(x, args["flip_mask"], args["crop_i"], args["crop_j"],
                        args["crop_h"], args["crop_w"], args["b_factor"],
                        args["c_factor"], args["s_factor"], args["h_factor"])
    key = (B_CORE, GPAIRS)
    if key not in _NC_CACHE:
        _NC_CACHE[key] = build_nc(B_CORE, GPAIRS)
    nc = _NC_CACHE[key]
    res = run_bass_kernel_spmd(nc, in_maps, list(range(N_CORES)))
    outs = [np.asarray(r["out"]) for r in res.results]
    return np.concatenate(outs, axis=0).astype(np.float32)


if __name__ == "__main__":
    nc = build_nc()
    print("built ok")



# revision 4
# speedup vs baseline: 1.7261x; 1.7261x over previous
"""Trainium2 Bass kernel for nn_DataAugmentation (flip + resized-crop +
brightness/contrast/saturation/hue) — 8-core data-parallel, v2.

Engine-balanced rewrite of the baseline:
- hat interp weights built on DVE (1 fused op per tile) instead of 2 ACT passes
- brightness eviction and contrast on ACT via negated-relu trick (upper clamp
  free): n1 = relu(1 - bf*T2), n2 = relu(cf*n1 + bias8); downstream custom ops
  un-negate for free
- contrast mean via DVE tensor_reduce (1 op) instead of 16 ACT accumulates
- hue: cre-scaled trapezoid (no reciprocal), sector select via telescoping
  mask blend in 2 fused DVE ops, per-channel out in 1 fused op + Pool subtract
- Pool (GpSimd) engine offloads: channel diffs, max-chain, blends, final subs
- output written contiguously per group; host un-permutes
"""

import numpy as np

import concourse.bass as bass
import concourse.bacc as bacc
import concourse.tile as tile
import concourse.mybir as mybir
from concourse.bass_utils import run_bass_kernel_spmd
from concourse.dve_spec import (
    Spec, Src0, Src1, C0, C1, C2, Zero, One, maxx, minn, Bin, AluOp, lower,
)
from concourse.dve_ops import DveOp, DveOpSpec, OPS, CUSTOM_DVE_SPECS, _SUB_OPCODE_FOR_NAME, has_src1

F32 = mybir.dt.float32
P = 128
OUT = 64
N_CORES = 8
B_FULL = 4096
B_CORE = B_FULL // N_CORES          # 512
GPAIRS = 16                         # pairs per group
NPAIR = B_CORE // 2                 # 256
NGROUP = NPAIR // GPAIRS            # 16
NFAC = 8
W_R = np.float32(0.2989)
W_G = np.float32(0.587)
W_B = np.float32(0.114)
WS = np.float32(W_R + W_G + W_B)


# ---------------------------------------------------------------- custom ops
def _register_op(name, spec):
    if name in _SUB_OPCODE_FOR_NAME:
        for o in OPS:
            if o.name == name:
                return o
    opc = 1 + len(OPS)
    _SUB_OPCODE_FOR_NAME[name] = opc
    shas = {}
    for ver in ("v3", "v4"):
        try:
            s = DveOpSpec(name=name, opcode=opc, uops=lower(spec, ver=ver),
                          rd1_en=has_src1(spec))
            shas[ver] = s.sha(ver)
        except ValueError:
            pass
    op = DveOp(name, spec, subdim=False, uops_sha=shas)
    OPS.append(op)
    CUSTOM_DVE_SPECS[name] = spec
    return op


def _refbc(v, like):
    if isinstance(v, np.ndarray) and v.ndim >= 1:
        return v.reshape(v.shape[0], *([1] * (like.ndim - 1))).astype(np.float32)
    return np.float32(v)


def _refsame(v, like):
    if v.shape == like.shape:
        return v
    if v.size == like.size:
        return v.reshape(like.shape)
    return np.broadcast_to(v.reshape(v.shape[0], 1, -1) if v.ndim == 2 else v, like.shape)


def _absd(a, b):
    return Bin(AluOp.ABSOLUTE_DIFF, a, b)


def _cl01(x):
    return np.clip(x, 0.0, 1.0).astype(np.float32)


# hat(x) = relu(1 - |x - c0|): bilinear interp weight (c0 = k+2 per partition)
HAT = _register_op("AG2_HAT", Spec(
    body=maxx(One - _absd(Src0, C0), Zero),
    reference=lambda in0, in1, s0, s1, imm2:
        np.maximum(1.0 - np.abs(in0 - _refbc(s0, in0)), 0.0).astype(np.float32),
))
# g = c0*in0 + c1*in1
G2 = _register_op("AG2_G2", Spec(
    body=C0 * Src0 + C1 * Src1,
    reference=lambda in0, in1, s0, s1, imm2:
        (np.float32(s0) * in0 + np.float32(s1) * _refsame(in1, in0)).astype(np.float32),
))
# g = in1 + c0*in0
G2ACC = _register_op("AG2_G2ACC", Spec(
    body=Src1 + C0 * Src0,
    reference=lambda in0, in1, s0, s1, imm2:
        (_refsame(in1, in0) + np.float32(s0) * in0).astype(np.float32),
))
# g = c0*cl01(in0) + c1*cl01(in1)
G2N = _register_op("AG2_G2N", Spec(
    body=C0 * minn(maxx(Src0, Zero), One) + C1 * minn(maxx(Src1, Zero), One),
    reference=lambda in0, in1, s0, s1, imm2:
        (np.float32(s0) * _cl01(in0) + np.float32(s1) * _cl01(_refsame(in1, in0))).astype(np.float32),
))
# g = in1 + c0*cl01(in0)
G2NACC = _register_op("AG2_G2NACC", Spec(
    body=Src1 + C0 * minn(maxx(Src0, Zero), One),
    reference=lambda in0, in1, s0, s1, imm2:
        (_refsame(in1, in0) + np.float32(s0) * _cl01(in0)).astype(np.float32),
))
# x3 = cl01(gb - c0*cl01(n2));  in0 = n2, in1 = gb (broadcast)
SATCL = _register_op("AG2_SATCL", Spec(
    body=minn(maxx(Src1 - C0 * minn(maxx(Src0, Zero), One), Zero), One),
    reference=lambda in0, in1, s0, s1, imm2:
        _cl01(_refsame(in1, in0) - _refbc(s0, in0) * _cl01(in0)),
))
# cre = max(|e1|, |e2|, |e1+e2|)
CREH = _register_op("AG2_CREH", Spec(
    body=maxx(maxx(_absd(Src0, Zero), _absd(Src1, Zero)), _absd(Src0, Zero - Src1)),
    reference=lambda in0, in1, s0, s1, imm2: np.maximum(
        np.maximum(np.abs(in0), np.abs(_refsame(in1, in0))),
        np.abs(in0 + _refsame(in1, in0))).astype(np.float32),
))


def _m1(e1, e2):
    s = e1 + e2
    return (s > Zero) | (e2 > Zero)          # not r-max


def _m2(e1, e2):
    return (e2 > Zero) & (e1 < Zero)         # b strictly max


def _refm1(in0, in1):
    return ((in0 + in1) > 0) | (in1 > 0)


def _refm2(in0, in1):
    return (in1 > 0) & (in0 < 0)


# t = e1 + (e2-e1)*m1
OPA = _register_op("AG2_OPA", Spec(
    body=Src0 + (Src1 - Src0) * _m1(Src0, Src1),
    reference=lambda in0, in1, s0, s1, imm2: (
        in0 + (_refsame(in1, in0) - in0) * _refm1(in0, _refsame(in1, in0))
    ).astype(np.float32),
))
# u = (e3-e2)*m2 = (-e1-2*e2)*m2
OPB = _register_op("AG2_OPB", Spec(
    body=(Zero - ((Src0 + Src1) + Src1)) * _m2(Src0, Src1),
    reference=lambda in0, in1, s0, s1, imm2: (
        (-(in0 + 2.0 * _refsame(in1, in0))) * _refm2(in0, _refsame(in1, in0))
    ).astype(np.float32),
))
# zh = m1 + m2   (zb/2)
ZBH = _register_op("AG2_ZBH", Spec(
    body=_m1(Src0, Src1) + _m2(Src0, Src1),
    reference=lambda in0, in1, s0, s1, imm2: (
        _refm1(in0, _refsame(in1, in0)).astype(np.float32)
        + _refm2(in0, _refsame(in1, in0)).astype(np.float32)
    ).astype(np.float32),
))
# wz = (zh + zh + c0) * cre   (c0 = hf6 per pair)
ZWH = _register_op("AG2_ZWH", Spec(
    body=((Src0 + Src0) + C0) * Src1,
    reference=lambda in0, in1, s0, s1, imm2:
        (((in0 + in0) + _refbc(s0, in0)) * _refsame(in1, in0)).astype(np.float32),
))
# qw = clamp(2*cre - ||v2 - c0*cre| - c1*cre|, 0, cre); in0 = v2, in1 = cre
QTRIM = _register_op("AG2_QTRIM", Spec(
    body=minn(maxx(Src1 + Src1 - _absd(_absd(Src0, C0 * Src1), C1 * Src1), Zero), Src1),
    reference=lambda in0, in1, s0, s1, imm2: (lambda cre: np.minimum(np.maximum(
        2.0 * cre - np.abs(np.abs(in0 - np.float32(s0) * cre) - np.float32(s1) * cre),
        0.0), cre))(_refsame(in1, in0)).astype(np.float32),
))


# ---------------------------------------------------------------- device program
def build_nc(b_core=B_CORE, gpairs=GPAIRS, debug=False):
    npair = b_core // 2
    ngroup = npair // gpairs
    assert ngroup * gpairs == npair
    G = gpairs
    FDP = OUT * G          # per-pixel free size per group (1024)
    AluT = mybir.AluOpType
    Act = mybir.ActivationFunctionType

    nc = bacc.Bacc("TRN2", target_bir_lowering=False, debug=debug)

    imgry_in = nc.dram_tensor("imgry", [ngroup, P, 512 * G], F32, kind="ExternalInput")
    rxfac_in = nc.dram_tensor("rxfac", [ngroup, P, (OUT + NFAC) * G], F32, kind="ExternalInput")
    o2_in = nc.dram_tensor("o2", [P, P], F32, kind="ExternalInput")
    out_d = nc.dram_tensor("out", [ngroup, P, 3 * FDP], F32, kind="ExternalOutput")

    with tile.TileContext(nc) as tc:
        with tc.tile_pool(name="persist", bufs=1) as pers, \
             tc.tile_pool(name="grp", bufs=2) as grp, \
             tc.tile_pool(name="hue", bufs=2) as hue, \
             tc.tile_pool(name="pp", bufs=2, space="PSUM") as pp:

            O2 = pers.tile([P, P], F32)
            nc.sync.dma_start(O2[:], o2_in[:])

            imgryd = [pers.tile([P, 512 * G], F32, tag=f"imgry{i}", name=f"imgry{i}") for i in range(3)]

            def front(g):
                eo = g % 3
                imgry = imgryd[eo]
                imgr = imgry[:, 0:384 * G].rearrange("p (gg c k) -> p gg c k", c=3, k=128)
                ryr = imgry[:, 384 * G:512 * G].rearrange("p (gg k) -> p gg k", k=128)

                # ---- input DMAs: host-prepacked block-diag img + hat weights
                nc.sync.dma_start(imgry[:], imgry_in[g])
                rxfac = grp.tile([P, (OUT + NFAC) * G], F32, tag="rxfac", bufs=3)
                rxr = rxfac[:, 0:OUT * G].rearrange("p (gg k) -> p gg k", k=OUT)
                nc.sync.dma_start(rxfac[:], rxfac_in[g])
                facr = rxfac[:, OUT * G:].rearrange("p (gg s) -> p gg s", s=NFAC)

                # ---- resize matmuls + eviction
                n1 = grp.tile([P, 192 * G], F32, tag="n1", bufs=3)
                n1r = n1[:].rearrange("p (gg c k) -> p gg c k", c=3, k=OUT)

                def stage2(b, sbt):
                    for half in range(2):
                        p = 2 * b + half
                        T2 = pp.tile([P, 192], F32, tag="t2", bufs=2)
                        for c in range(3):
                            nc.tensor.matmul(
                                T2[:, 64 * c:64 * (c + 1)],
                                sbt[:, 384 * half + 128 * c:384 * half + 128 * (c + 1)],
                                rxr[:, p, :], start=True, stop=True)
                        # n1 = relu(1 - bf*T2)   (negated brightness, clamped)
                        nc.scalar.activation(
                            n1[:, 192 * p:192 * (p + 1)], T2[:], Act.Relu,
                            bias=1.0, scale=facr[:, p, 0:1])

                prev = None
                for b in range(G // 2):
                    T1D = pp.tile([P, 768], F32, tag="t1", bufs=2)
                    for half in range(2):
                        p = 2 * b + half
                        for c in range(3):
                            nc.tensor.matmul(
                                T1D[:, 384 * half + 128 * c:384 * half + 128 * (c + 1)],
                                imgr[:, p, c, :], ryr[:, p, :],
                                start=True, stop=True)
                    sbt = grp.tile([P, 768], F32, tag="sbt", bufs=3)
                    nc.scalar.copy(sbt[:], T1D[:])
                    if prev is not None:
                        stage2(*prev)
                    prev = (b, sbt)
                stage2(*prev)

                # ---- contrast mean path: g1n = sum w_c * n1_c ; tb from sums
                g1n = grp.tile([P, FDP], F32, tag="gsum")
                g1r = g1n[:].rearrange("p (gg k) -> p gg k", k=OUT)
                nc.vector._custom_dve(G2, out=g1r[:, :, :], in0=n1r[:, :, 0, :],
                                      in1=n1r[:, :, 1, :], s0=float(W_R), s1=float(W_G))
                nc.vector._custom_dve(G2ACC, out=g1r[:, :, :], in0=n1r[:, :, 2, :],
                                      in1=g1r[:, :, :], s0=float(W_B))
                mrow = grp.tile([P, G], F32, tag="mrow")
                nc.vector.tensor_reduce(mrow[:], g1r[:, :, :],
                                        mybir.AxisListType.X, AluT.add)
                Mcol = pp.tile([P, G], F32, tag="mcol", bufs=1)
                nc.tensor.matmul(Mcol[:], O2[:], mrow[:], start=True, stop=True)
                bias8 = grp.tile([P, G], F32, tag="bias8")
                nc.vector.tensor_tensor(bias8[:], Mcol[:], facr[:, :, 5], AluT.mult)
                nc.vector.tensor_tensor(bias8[:], bias8[:], facr[:, :, 6], AluT.add)

                return dict(g=g, facr=facr, n1=n1, n1r=n1r,
                            g1n=g1n, g1r=g1r, bias8=bias8)

            def mid(st):
                g = st["g"]
                facr = st["facr"]
                n1 = st["n1"]
                n1r = st["n1r"]
                g1n = st["g1n"]
                g1r = st["g1r"]
                bias8 = st["bias8"]

                # ---- contrast: n2 = relu(cf*n1 + bias8) in place
                for p in range(G):
                    nc.scalar.activation(
                        n1[:, 192 * p:192 * (p + 1)], n1[:, 192 * p:192 * (p + 1)],
                        Act.Relu, bias=bias8[:, p:p + 1], scale=facr[:, p, 1:2])

                # ---- saturation: g2n, gb (Pool), x3 (in place on n1/n2)
                nc.vector._custom_dve(G2N, out=g1r[:, :, :], in0=n1r[:, :, 0, :],
                                      in1=n1r[:, :, 1, :], s0=float(W_R), s1=float(W_G))
                nc.vector._custom_dve(G2NACC, out=g1r[:, :, :], in0=n1r[:, :, 2, :],
                                      in1=g1r[:, :, :], s0=float(W_B))
                gb = hue.tile([P, FDP], F32, tag="gb", bufs=2)
                for p in range(G):
                    nc.scalar.activation(
                        gb[:, OUT * p:OUT * (p + 1)], g1n[:, OUT * p:OUT * (p + 1)],
                        Act.Identity, bias=facr[:, p, 4:5], scale=facr[:, p, 3:4])
                for p in range(G):
                    nc.vector._custom_dve(
                        SATCL, out=n1r[:, p, :, :], in0=n1r[:, p, :, :],
                        in1=gb[:, OUT * p:OUT * (p + 1)].unsqueeze(1).broadcast_to((P, 3, OUT)),
                        s0=facr[:, p, 2:3])

            def tail(st):
                g = st["g"]
                facr = st["facr"]
                n1 = st["n1"]
                n1r = st["n1r"]
                x3r = n1r

                # ---- hue
                r_s = x3r[:, :, 0, :]
                g_s = x3r[:, :, 1, :]
                b_s = x3r[:, :, 2, :]
                e1 = hue.tile([P, FDP], F32, tag="e1", bufs=1)
                e2 = hue.tile([P, FDP], F32, tag="e2", bufs=1)
                cre = hue.tile([P, FDP], F32, tag="cre", bufs=1)
                ta = hue.tile([P, FDP], F32, tag="ta", bufs=1)
                ub = hue.tile([P, FDP], F32, tag="ub", bufs=1)
                zh = hue.tile([P, FDP], F32, tag="zh", bufs=1)
                hfc = hue.tile([P, FDP], F32, tag="hfc", bufs=1)
                v2 = hue.tile([P, FDP], F32, tag="v2", bufs=1)
                mxt = hue.tile([P, FDP], F32, tag="e2", bufs=1)
                e1r = e1[:].rearrange("p (gg k) -> p gg k", k=OUT)
                e2r = e2[:].rearrange("p (gg k) -> p gg k", k=OUT)
                mxr = mxt[:].rearrange("p (gg k) -> p gg k", k=OUT)

                nc.vector.tensor_tensor(e1r[:, :, :], g_s, b_s, AluT.subtract)
                nc.vector.tensor_tensor(e2r[:, :, :], b_s, r_s, AluT.subtract)
                nc.vector._custom_dve(CREH, out=cre[:], in0=e1[:], in1=e2[:])
                nc.vector._custom_dve(OPA, out=ta[:], in0=e1[:], in1=e2[:])
                nc.vector._custom_dve(OPB, out=ub[:], in0=e1[:], in1=e2[:])
                nc.vector._custom_dve(ZBH, out=zh[:], in0=e1[:], in1=e2[:])
                nc.vector.tensor_tensor(ta[:], ta[:], ub[:], AluT.add)   # esel
                # wz = (2*zh + hf6) * cre (per pair)
                for p in range(G):
                    nc.vector._custom_dve(
                        ZWH, out=hfc[:, OUT * p:OUT * (p + 1)],
                        in0=zh[:, OUT * p:OUT * (p + 1)],
                        in1=cre[:, OUT * p:OUT * (p + 1)],
                        s0=facr[:, p, 7:8])
                nc.vector.tensor_tensor(v2[:], ta[:], hfc[:], AluT.add)
                nc.vector.tensor_tensor(mxr[:, :, :], r_s, g_s, AluT.max)
                nc.vector.tensor_tensor(mxr[:, :, :], mxr[:, :, :], b_s, AluT.max)
                for ci, cen in enumerate((0.0, 2.0, 4.0)):
                    qt = hue.tile([P, FDP], F32, tag="qt", bufs=1)
                    nc.vector._custom_dve(QTRIM, out=qt[:], in0=v2[:], in1=cre[:],
                                          s0=cen, s1=3.0)
                    nc.vector.tensor_tensor(x3r[:, :, ci, :],
                                            mxr[:, :, :],
                                            qt[:].rearrange("p (gg k) -> p gg k", k=OUT),
                                            AluT.subtract)

                nc.gpsimd.dma_start(out_d[g], n1[:])

            sts = {}
            for g in range(ngroup):
                if g >= 2:
                    tail(sts.pop(g - 2))
                if g >= 1:
                    mid(sts[g - 1])
                sts[g] = front(g)
            mid(sts[ngroup - 1])
            tail(sts.pop(ngroup - 2))
            tail(sts.pop(ngroup - 1))

    nc.compile()
    return nc


# ---------------------------------------------------------------- host prep
def host_prep(x, flip_mask, crop_i, crop_j, crop_h, crop_w,
              b_factor, c_factor, s_factor, h_factor,
              b_core=B_CORE, gpairs=GPAIRS):
    f32 = np.float32
    B = x.shape[0]
    npair = b_core // 2
    ngroup = npair // gpairs
    G = gpairs

    ar = (np.arange(OUT, dtype=f32) + f32(0.5))
    ys = crop_i[:, None].astype(f32) + ar[None, :] * (crop_h.astype(f32)[:, None] / f32(OUT)) - f32(0.5)
    xs = crop_j[:, None].astype(f32) + ar[None, :] * (crop_w.astype(f32)[:, None] / f32(OUT)) - f32(0.5)

    def eff(p):
        return np.where(p < 0, p + f32(1.0), np.minimum(p, f32(63.0))).astype(f32)

    ysv = eff(ys)
    xsv = np.where(flip_mask[:, None], f32(63.0) - eff(xs), eff(xs)).astype(f32)

    bf = b_factor.astype(f32)
    cf = c_factor.astype(f32)
    sf = s_factor.astype(f32)
    osf = (f32(1.0) - sf).astype(f32)

    negbf = (-bf).astype(f32)
    nosf = (-osf).astype(f32)
    c2sat = (sf + osf * WS).astype(f32)
    cbv = ((f32(1.0) - cf) / f32(4096.0)).astype(f32)
    hc1 = (f32(1.0) - cf - (f32(1.0) - cf) * WS).astype(f32)
    hf6 = (f32(6.0) * h_factor.astype(f32)).astype(f32)

    o2 = np.zeros((P, P), dtype=f32)
    o2[:64, :64] = 1.0
    o2[64:, 64:] = 1.0

    rows = np.arange(64, dtype=f32)
    # hat weights per sample: w[b, in, out] = relu(1 - |pos[b, out] - in|)
    wy = np.maximum(f32(0.0), f32(1.0) - np.abs(ysv[:, None, :] - rows[None, :, None])).astype(f32)
    wx = np.maximum(f32(0.0), f32(1.0) - np.abs(xsv[:, None, :] - rows[None, :, None])).astype(f32)

    per_core = []
    n_cores = B // b_core
    for k in range(n_cores):
        sl = slice(k * b_core, (k + 1) * b_core)
        fac = np.zeros((ngroup, P, NFAC * G), dtype=f32)
        vals = np.stack([negbf[sl], cf[sl], sf[sl], nosf[sl],
                         c2sat[sl], cbv[sl], hc1[sl], hf6[sl]], -1)  # [b_core, 8]
        vals = vals.reshape(ngroup, G, 2, NFAC)
        for s, rws in ((0, slice(0, 64)), (1, slice(64, 128))):
            v = vals[:, :, s, :].reshape(ngroup, 1, G * NFAC)
            fac[:, rws, :] = np.broadcast_to(v, (ngroup, 64, G * NFAC))

        # block-diag image: [g, (s,row) 128, gg, c, (s', col) 128]
        xc = x[sl].astype(f32).reshape(ngroup, G, 2, 3, 64, 64)  # g, gg, s, c, row, col
        xblk = np.zeros((ngroup, 2, 64, G, 3, 2, 64), dtype=f32)  # g, s, row, gg, c, s', col
        xblk[:, 0, :, :, :, 0, :] = xc[:, :, 0].transpose(0, 3, 1, 2, 4)  # A rows, A cols
        xblk[:, 1, :, :, :, 1, :] = xc[:, :, 1].transpose(0, 3, 1, 2, 4)  # B rows, B cols
        xblk = xblk.reshape(ngroup, P, 384 * G)

        # ry weights: [g, (s,in-row) 128, gg, (s', out-row) 128], diag blocks only
        wyc = wy[sl].reshape(ngroup, G, 2, 64, 64)               # g, gg, s, in, out
        ryw = np.zeros((ngroup, 2, 64, G, 2, 64), dtype=f32)     # g, s, in, gg, s', out
        ryw[:, 0, :, :, 0, :] = wyc[:, :, 0].transpose(0, 2, 1, 3)
        ryw[:, 1, :, :, 1, :] = wyc[:, :, 1].transpose(0, 2, 1, 3)
        ryw = ryw.reshape(ngroup, P, 128 * G)

        # rx weights: [g, (s,col) 128, gg, out-col 64] (dense, sample by half)
        wxc = wx[sl].reshape(ngroup, G, 2, 64, 64)               # g, gg, s, col, out
        rxw = wxc.transpose(0, 2, 3, 1, 4).reshape(ngroup, P, OUT * G)

        imgry = np.concatenate([xblk, ryw], axis=2)
        rxfac = np.concatenate([rxw, fac], axis=2)
        per_core.append({
            "imgry": np.ascontiguousarray(imgry),
            "rxfac": np.ascontiguousarray(rxfac),
            "o2": o2,
        })
    return per_core


def unpermute(out_arr, b_core=B_CORE, gpairs=GPAIRS):
    """[ngroup, 128, 3*G*64] device layout -> [b_core, 3, 64, 64]."""
    ngroup = (b_core // 2) // gpairs
    o = out_arr.reshape(ngroup, 2, 64, gpairs, 3, 64)     # g, s, r, gg, c, col
    o = o.transpose(0, 3, 1, 4, 2, 5)                      # g, gg, s, c, r, col
    return np.ascontiguousarray(o.reshape(b_core, 3, 64, 64))


_NC_CACHE = {}


def kernel(**inputs):
    x = np.asarray(inputs["x"], dtype=np.float32)
    args = {k: np.asarray(inputs[k]) for k in
            ("flip_mask", "crop_i", "crop_j", "crop_h", "crop_w",
             "b_factor", "c_factor", "s_factor", "h_factor")}
    in_maps = host_prep(x, args["flip_mask"], args["crop_i"], args["crop_j"],
                        args["crop_h"], args["crop_w"], args["b_factor"],
                        args["c_factor"], args["s_factor"], args["h_factor"])
    key = (B_CORE, GPAIRS)
    if key not in _NC_CACHE:
        _NC_CACHE[key] = build_nc(B_CORE, GPAIRS)
    nc = _NC_CACHE[key]
    res = run_bass_kernel_spmd(nc, in_maps, list(range(N_CORES)))
    outs = [unpermute(np.asarray(r["out"])) for r in res.results]
    return np.concatenate(outs, axis=0).astype(np.float32)


if __name__ == "__main__":
    nc = build_nc()
    print("built ok")


# revision 5
# speedup vs baseline: 1.7714x; 1.0263x over previous
"""Trainium2 Bass kernel for nn_DataAugmentation (flip + resized-crop +
brightness/contrast/saturation/hue) — 8-core data-parallel, v2.

Engine-balanced rewrite of the baseline:
- hat interp weights built on DVE (1 fused op per tile) instead of 2 ACT passes
- brightness eviction and contrast on ACT via negated-relu trick (upper clamp
  free): n1 = relu(1 - bf*T2), n2 = relu(cf*n1 + bias8); downstream custom ops
  un-negate for free
- contrast mean via DVE tensor_reduce (1 op) instead of 16 ACT accumulates
- hue: cre-scaled trapezoid (no reciprocal), sector select via telescoping
  mask blend in 2 fused DVE ops, per-channel out in 1 fused op + Pool subtract
- Pool (GpSimd) engine offloads: channel diffs, max-chain, blends, final subs
- output written contiguously per group; host un-permutes
"""

import numpy as np

import concourse.bass as bass
import concourse.bacc as bacc
import concourse.tile as tile
import concourse.mybir as mybir
from concourse.bass_utils import run_bass_kernel_spmd
from concourse.dve_spec import (
    Spec, Src0, Src1, C0, C1, C2, Zero, One, maxx, minn, Bin, AluOp, lower,
)
from concourse.dve_ops import DveOp, DveOpSpec, OPS, CUSTOM_DVE_SPECS, _SUB_OPCODE_FOR_NAME, has_src1

F32 = mybir.dt.float32
P = 128
OUT = 64
N_CORES = 8
B_FULL = 4096
B_CORE = B_FULL // N_CORES          # 512
GPAIRS = 16                         # pairs per group
NPAIR = B_CORE // 2                 # 256
NGROUP = NPAIR // GPAIRS            # 16
NFAC = 8
W_R = np.float32(0.2989)
W_G = np.float32(0.587)
W_B = np.float32(0.114)
WS = np.float32(W_R + W_G + W_B)


# ---------------------------------------------------------------- custom ops
def _register_op(name, spec):
    if name in _SUB_OPCODE_FOR_NAME:
        for o in OPS:
            if o.name == name:
                return o
    opc = 1 + len(OPS)
    _SUB_OPCODE_FOR_NAME[name] = opc
    shas = {}
    for ver in ("v3", "v4"):
        try:
            s = DveOpSpec(name=name, opcode=opc, uops=lower(spec, ver=ver),
                          rd1_en=has_src1(spec))
            shas[ver] = s.sha(ver)
        except ValueError:
            pass
    op = DveOp(name, spec, subdim=False, uops_sha=shas)
    OPS.append(op)
    CUSTOM_DVE_SPECS[name] = spec
    return op


def _refbc(v, like):
    if isinstance(v, np.ndarray) and v.ndim >= 1:
        return v.reshape(v.shape[0], *([1] * (like.ndim - 1))).astype(np.float32)
    return np.float32(v)


def _refsame(v, like):
    if v.shape == like.shape:
        return v
    if v.size == like.size:
        return v.reshape(like.shape)
    return np.broadcast_to(v.reshape(v.shape[0], 1, -1) if v.ndim == 2 else v, like.shape)


def _absd(a, b):
    return Bin(AluOp.ABSOLUTE_DIFF, a, b)


def _cl01(x):
    return np.clip(x, 0.0, 1.0).astype(np.float32)


# hat(x) = relu(1 - |x - c0|): bilinear interp weight (c0 = k+2 per partition)
HAT = _register_op("AG2_HAT", Spec(
    body=maxx(One - _absd(Src0, C0), Zero),
    reference=lambda in0, in1, s0, s1, imm2:
        np.maximum(1.0 - np.abs(in0 - _refbc(s0, in0)), 0.0).astype(np.float32),
))
# g = c0*in0 + c1*in1
G2 = _register_op("AG2_G2", Spec(
    body=C0 * Src0 + C1 * Src1,
    reference=lambda in0, in1, s0, s1, imm2:
        (np.float32(s0) * in0 + np.float32(s1) * _refsame(in1, in0)).astype(np.float32),
))
# g = in1 + c0*in0
G2ACC = _register_op("AG2_G2ACC", Spec(
    body=Src1 + C0 * Src0,
    reference=lambda in0, in1, s0, s1, imm2:
        (_refsame(in1, in0) + np.float32(s0) * in0).astype(np.float32),
))
# g = c0*cl01(in0) + c1*cl01(in1)
G2N = _register_op("AG2_G2N", Spec(
    body=C0 * minn(maxx(Src0, Zero), One) + C1 * minn(maxx(Src1, Zero), One),
    reference=lambda in0, in1, s0, s1, imm2:
        (np.float32(s0) * _cl01(in0) + np.float32(s1) * _cl01(_refsame(in1, in0))).astype(np.float32),
))
# g = in1 + c0*cl01(in0)
G2NACC = _register_op("AG2_G2NACC", Spec(
    body=Src1 + C0 * minn(maxx(Src0, Zero), One),
    reference=lambda in0, in1, s0, s1, imm2:
        (_refsame(in1, in0) + np.float32(s0) * _cl01(in0)).astype(np.float32),
))
# x3 = cl01(gb - c0*cl01(n2));  in0 = n2, in1 = gb (broadcast)
SATCL = _register_op("AG2_SATCL", Spec(
    body=minn(maxx(Src1 - C0 * minn(maxx(Src0, Zero), One), Zero), One),
    reference=lambda in0, in1, s0, s1, imm2:
        _cl01(_refsame(in1, in0) - _refbc(s0, in0) * _cl01(in0)),
))
# cre = max(|e1|, |e2|, |e1+e2|)
CREH = _register_op("AG2_CREH", Spec(
    body=maxx(maxx(_absd(Src0, Zero), _absd(Src1, Zero)), _absd(Src0, Zero - Src1)),
    reference=lambda in0, in1, s0, s1, imm2: np.maximum(
        np.maximum(np.abs(in0), np.abs(_refsame(in1, in0))),
        np.abs(in0 + _refsame(in1, in0))).astype(np.float32),
))


def _m1(e1, e2):
    s = e1 + e2
    return (s > Zero) | (e2 > Zero)          # not r-max


def _m2(e1, e2):
    return (e2 > Zero) & (e1 < Zero)         # b strictly max


def _refm1(in0, in1):
    return ((in0 + in1) > 0) | (in1 > 0)


def _refm2(in0, in1):
    return (in1 > 0) & (in0 < 0)


# t = e1 + (e2-e1)*m1
OPA = _register_op("AG2_OPA", Spec(
    body=Src0 + (Src1 - Src0) * _m1(Src0, Src1),
    reference=lambda in0, in1, s0, s1, imm2: (
        in0 + (_refsame(in1, in0) - in0) * _refm1(in0, _refsame(in1, in0))
    ).astype(np.float32),
))
# u = (e3-e2)*m2 = (-e1-2*e2)*m2
OPB = _register_op("AG2_OPB", Spec(
    body=(Zero - ((Src0 + Src1) + Src1)) * _m2(Src0, Src1),
    reference=lambda in0, in1, s0, s1, imm2: (
        (-(in0 + 2.0 * _refsame(in1, in0))) * _refm2(in0, _refsame(in1, in0))
    ).astype(np.float32),
))
# zh = m1 + m2   (zb/2)
ZBH = _register_op("AG2_ZBH", Spec(
    body=_m1(Src0, Src1) + _m2(Src0, Src1),
    reference=lambda in0, in1, s0, s1, imm2: (
        _refm1(in0, _refsame(in1, in0)).astype(np.float32)
        + _refm2(in0, _refsame(in1, in0)).astype(np.float32)
    ).astype(np.float32),
))
# wz = (zh + zh + c0) * cre   (c0 = hf6 per pair)
ZWH = _register_op("AG2_ZWH", Spec(
    body=((Src0 + Src0) + C0) * Src1,
    reference=lambda in0, in1, s0, s1, imm2:
        (((in0 + in0) + _refbc(s0, in0)) * _refsame(in1, in0)).astype(np.float32),
))
# qw = clamp(2*cre - ||v2 - c0*cre| - c1*cre|, 0, cre); in0 = v2, in1 = cre
QTRIM = _register_op("AG2_QTRIM", Spec(
    body=minn(maxx(Src1 + Src1 - _absd(_absd(Src0, C0 * Src1), C1 * Src1), Zero), Src1),
    reference=lambda in0, in1, s0, s1, imm2: (lambda cre: np.minimum(np.maximum(
        2.0 * cre - np.abs(np.abs(in0 - np.float32(s0) * cre) - np.float32(s1) * cre),
        0.0), cre))(_refsame(in1, in0)).astype(np.float32),
))


# ---------------------------------------------------------------- device program
def build_nc(b_core=B_CORE, gpairs=GPAIRS, debug=False):
    npair = b_core // 2
    ngroup = npair // gpairs
    assert ngroup * gpairs == npair
    G = gpairs
    FDP = OUT * G          # per-pixel free size per group (1024)
    AluT = mybir.AluOpType
    Act = mybir.ActivationFunctionType

    nc = bacc.Bacc("TRN2", target_bir_lowering=False, debug=debug)

    imgry_in = nc.dram_tensor("imgry", [ngroup, P, 512 * G], F32, kind="ExternalInput")
    rxfac_in = nc.dram_tensor("rxfac", [ngroup, P, (OUT + NFAC) * G], F32, kind="ExternalInput")
    o2_in = nc.dram_tensor("o2", [P, P], F32, kind="ExternalInput")
    out_d = nc.dram_tensor("out", [ngroup, P, 3 * FDP], F32, kind="ExternalOutput")

    with tile.TileContext(nc) as tc:
        with tc.tile_pool(name="persist", bufs=1) as pers, \
             tc.tile_pool(name="grp", bufs=2) as grp, \
             tc.tile_pool(name="hue", bufs=2) as hue, \
             tc.tile_pool(name="pp", bufs=2, space="PSUM") as pp:

            O2 = pers.tile([P, P], F32)
            nc.sync.dma_start(O2[:], o2_in[:])

            imgryd = [pers.tile([P, 512 * G], F32, tag=f"imgry{i}", name=f"imgry{i}") for i in range(3)]

            def front(g, p0, gs, ui):
                p1 = p0 + gs
                imgry = imgryd[ui % 3]
                imgr = imgry[:, 0:384 * G].rearrange("p (gg c k) -> p gg c k", c=3, k=128)
                ryr = imgry[:, 384 * G:512 * G].rearrange("p (gg k) -> p gg k", k=128)

                # ---- input DMAs: host-prepacked block-diag img + hat weights
                if gs == G:
                    nc.sync.dma_start(imgry[:], imgry_in[g])
                else:
                    nc.sync.dma_start(imgry[:, 384 * p0:384 * p1],
                                      imgry_in[g][:, 384 * p0:384 * p1])
                    nc.sync.dma_start(imgry[:, 384 * G + 128 * p0:384 * G + 128 * p1],
                                      imgry_in[g][:, 384 * G + 128 * p0:384 * G + 128 * p1])
                rxfac = grp.tile([P, (OUT + NFAC) * G], F32, tag="rxfac", bufs=3)
                rxr = rxfac[:, 0:OUT * G].rearrange("p (gg k) -> p gg k", k=OUT)
                if gs == G:
                    nc.sync.dma_start(rxfac[:], rxfac_in[g])
                else:
                    nc.sync.dma_start(rxfac[:, OUT * p0:OUT * p1],
                                      rxfac_in[g][:, OUT * p0:OUT * p1])
                    nc.sync.dma_start(rxfac[:, OUT * G + NFAC * p0:OUT * G + NFAC * p1],
                                      rxfac_in[g][:, OUT * G + NFAC * p0:OUT * G + NFAC * p1])
                facr = rxfac[:, OUT * G:].rearrange("p (gg s) -> p gg s", s=NFAC)

                # ---- resize matmuls + eviction
                n1 = grp.tile([P, 192 * G], F32, tag="n1", bufs=3)
                n1r = n1[:].rearrange("p (gg c k) -> p gg c k", c=3, k=OUT)

                def stage2(b, sbt):
                    for half in range(2):
                        p = 2 * b + half
                        T2 = pp.tile([P, 192], F32, tag="t2", bufs=2)
                        for c in range(3):
                            nc.tensor.matmul(
                                T2[:, 64 * c:64 * (c + 1)],
                                sbt[:, 384 * half + 128 * c:384 * half + 128 * (c + 1)],
                                rxr[:, p, :], start=True, stop=True)
                        # n1 = relu(1 - bf*T2)   (negated brightness, clamped)
                        nc.scalar.activation(
                            n1[:, 192 * p:192 * (p + 1)], T2[:], Act.Relu,
                            bias=1.0, scale=facr[:, p, 0:1])

                prev = None
                for b in range(p0 // 2, p1 // 2):
                    T1D = pp.tile([P, 768], F32, tag="t1", bufs=2)
                    for half in range(2):
                        p = 2 * b + half
                        for c in range(3):
                            nc.tensor.matmul(
                                T1D[:, 384 * half + 128 * c:384 * half + 128 * (c + 1)],
                                imgr[:, p, c, :], ryr[:, p, :],
                                start=True, stop=True)
                    sbt = grp.tile([P, 768], F32, tag="sbt", bufs=3)
                    nc.scalar.copy(sbt[:], T1D[:])
                    if prev is not None:
                        stage2(*prev)
                    prev = (b, sbt)
                stage2(*prev)

                # ---- contrast mean path: g1n = sum w_c * n1_c ; tb from sums
                g1n = grp.tile([P, FDP], F32, tag="gsum")
                g1r = g1n[:].rearrange("p (gg k) -> p gg k", k=OUT)
                nc.vector._custom_dve(G2, out=g1r[:, p0:p1, :], in0=n1r[:, p0:p1, 0, :],
                                      in1=n1r[:, p0:p1, 1, :], s0=float(W_R), s1=float(W_G))
                nc.vector._custom_dve(G2ACC, out=g1r[:, p0:p1, :], in0=n1r[:, p0:p1, 2, :],
                                      in1=g1r[:, p0:p1, :], s0=float(W_B))
                mrow = grp.tile([P, G], F32, tag="mrow")
                nc.vector.tensor_reduce(mrow[:, p0:p1], g1r[:, p0:p1, :],
                                        mybir.AxisListType.X, AluT.add)
                Mcol = pp.tile([P, G], F32, tag="mcol", bufs=1)
                nc.tensor.matmul(Mcol[:, p0:p1], O2[:], mrow[:, p0:p1], start=True, stop=True)
                bias8 = grp.tile([P, G], F32, tag="bias8")
                nc.vector.tensor_tensor(bias8[:, p0:p1], Mcol[:, p0:p1],
                                        facr[:, p0:p1, 5], AluT.mult)
                nc.vector.tensor_tensor(bias8[:, p0:p1], bias8[:, p0:p1],
                                        facr[:, p0:p1, 6], AluT.add)

                return dict(g=g, p0=p0, p1=p1, facr=facr, n1=n1, n1r=n1r,
                            g1n=g1n, g1r=g1r, bias8=bias8)

            def mid(st):
                g = st["g"]
                p0 = st["p0"]
                p1 = st["p1"]
                facr = st["facr"]
                n1 = st["n1"]
                n1r = st["n1r"]
                g1n = st["g1n"]
                g1r = st["g1r"]
                bias8 = st["bias8"]

                # ---- contrast: n2 = relu(cf*n1 + bias8) in place
                for p in range(p0, p1):
                    nc.scalar.activation(
                        n1[:, 192 * p:192 * (p + 1)], n1[:, 192 * p:192 * (p + 1)],
                        Act.Relu, bias=bias8[:, p:p + 1], scale=facr[:, p, 1:2])

                # ---- saturation: g2n, gb (Pool), x3 (in place on n1/n2)
                nc.vector._custom_dve(G2N, out=g1r[:, p0:p1, :], in0=n1r[:, p0:p1, 0, :],
                                      in1=n1r[:, p0:p1, 1, :], s0=float(W_R), s1=float(W_G))
                nc.vector._custom_dve(G2NACC, out=g1r[:, p0:p1, :], in0=n1r[:, p0:p1, 2, :],
                                      in1=g1r[:, p0:p1, :], s0=float(W_B))
                gb = hue.tile([P, FDP], F32, tag="gb", bufs=2)
                for p in range(p0, p1):
                    nc.scalar.activation(
                        gb[:, OUT * p:OUT * (p + 1)], g1n[:, OUT * p:OUT * (p + 1)],
                        Act.Identity, bias=facr[:, p, 4:5], scale=facr[:, p, 3:4])
                for p in range(p0, p1):
                    nc.vector._custom_dve(
                        SATCL, out=n1r[:, p, :, :], in0=n1r[:, p, :, :],
                        in1=gb[:, OUT * p:OUT * (p + 1)].unsqueeze(1).broadcast_to((P, 3, OUT)),
                        s0=facr[:, p, 2:3])

            def tail(st):
                g = st["g"]
                p0 = st["p0"]
                p1 = st["p1"]
                fl0 = OUT * p0
                fl1 = OUT * p1
                facr = st["facr"]
                n1 = st["n1"]
                n1r = st["n1r"]
                x3r = n1r

                # ---- hue
                r_s = x3r[:, p0:p1, 0, :]
                g_s = x3r[:, p0:p1, 1, :]
                b_s = x3r[:, p0:p1, 2, :]
                e1 = hue.tile([P, FDP], F32, tag="e1", bufs=1)
                e2 = hue.tile([P, FDP], F32, tag="e2", bufs=1)
                cre = hue.tile([P, FDP], F32, tag="cre", bufs=1)
                ta = hue.tile([P, FDP], F32, tag="ta", bufs=1)
                ub = hue.tile([P, FDP], F32, tag="ub", bufs=1)
                zh = hue.tile([P, FDP], F32, tag="zh", bufs=1)
                hfc = hue.tile([P, FDP], F32, tag="hfc", bufs=1)
                v2 = hue.tile([P, FDP], F32, tag="v2", bufs=1)
                mxt = hue.tile([P, FDP], F32, tag="e2", bufs=1)
                e1r = e1[:].rearrange("p (gg k) -> p gg k", k=OUT)
                e2r = e2[:].rearrange("p (gg k) -> p gg k", k=OUT)
                mxr = mxt[:].rearrange("p (gg k) -> p gg k", k=OUT)

                nc.vector.tensor_tensor(e1r[:, p0:p1, :], g_s, b_s, AluT.subtract)
                nc.vector.tensor_tensor(e2r[:, p0:p1, :], b_s, r_s, AluT.subtract)
                nc.vector._custom_dve(CREH, out=cre[:, fl0:fl1], in0=e1[:, fl0:fl1], in1=e2[:, fl0:fl1])
                nc.vector._custom_dve(OPA, out=ta[:, fl0:fl1], in0=e1[:, fl0:fl1], in1=e2[:, fl0:fl1])
                nc.vector._custom_dve(OPB, out=ub[:, fl0:fl1], in0=e1[:, fl0:fl1], in1=e2[:, fl0:fl1])
                nc.vector._custom_dve(ZBH, out=zh[:, fl0:fl1], in0=e1[:, fl0:fl1], in1=e2[:, fl0:fl1])
                nc.vector.tensor_tensor(ta[:, fl0:fl1], ta[:, fl0:fl1], ub[:, fl0:fl1], AluT.add)   # esel
                # wz = (2*zh + hf6) * cre (per pair)
                for p in range(p0, p1):
                    nc.vector._custom_dve(
                        ZWH, out=hfc[:, OUT * p:OUT * (p + 1)],
                        in0=zh[:, OUT * p:OUT * (p + 1)],
                        in1=cre[:, OUT * p:OUT * (p + 1)],
                        s0=facr[:, p, 7:8])
                nc.vector.tensor_tensor(v2[:, fl0:fl1], ta[:, fl0:fl1], hfc[:, fl0:fl1], AluT.add)
                nc.vector.tensor_tensor(mxr[:, p0:p1, :], r_s, g_s, AluT.max)
                nc.vector.tensor_tensor(mxr[:, p0:p1, :], mxr[:, p0:p1, :], b_s, AluT.max)
                for ci, cen in enumerate((0.0, 2.0, 4.0)):
                    qt = hue.tile([P, FDP], F32, tag="qt", bufs=1)
                    nc.vector._custom_dve(QTRIM, out=qt[:, fl0:fl1], in0=v2[:, fl0:fl1],
                                          in1=cre[:, fl0:fl1], s0=cen, s1=3.0)
                    nc.vector.tensor_tensor(x3r[:, p0:p1, ci, :],
                                            mxr[:, p0:p1, :],
                                            qt[:, fl0:fl1].rearrange("p (gg k) -> p gg k", k=OUT),
                                            AluT.subtract)

                nc.gpsimd.dma_start(out_d[g][:, 192 * p0:192 * p1],
                                    n1[:, 192 * p0:192 * p1])

            units = ([(0, 0, 4), (0, 4, 4), (0, 8, 8)]
                     + [(g, 0, G) for g in range(1, ngroup - 1)]
                     + [(ngroup - 1, 0, 8), (ngroup - 1, 8, 4), (ngroup - 1, 12, 4)])
            nu = len(units)
            sts = {}
            for i, (g, p0, gs) in enumerate(units):
                if i >= 2:
                    tail(sts.pop(i - 2))
                if i >= 1:
                    mid(sts[i - 1])
                sts[i] = front(g, p0, gs, i)
            mid(sts[nu - 1])
            tail(sts.pop(nu - 2))
            tail(sts.pop(nu - 1))

    nc.compile()
    return nc


# ---------------------------------------------------------------- host prep
def host_prep(x, flip_mask, crop_i, crop_j, crop_h, crop_w,
              b_factor, c_factor, s_factor, h_factor,
              b_core=B_CORE, gpairs=GPAIRS):
    f32 = np.float32
    B = x.shape[0]
    npair = b_core // 2
    ngroup = npair // gpairs
    G = gpairs

    ar = (np.arange(OUT, dtype=f32) + f32(0.5))
    ys = crop_i[:, None].astype(f32) + ar[None, :] * (crop_h.astype(f32)[:, None] / f32(OUT)) - f32(0.5)
    xs = crop_j[:, None].astype(f32) + ar[None, :] * (crop_w.astype(f32)[:, None] / f32(OUT)) - f32(0.5)

    def eff(p):
        return np.where(p < 0, p + f32(1.0), np.minimum(p, f32(63.0))).astype(f32)

    ysv = eff(ys)
    xsv = np.where(flip_mask[:, None], f32(63.0) - eff(xs), eff(xs)).astype(f32)

    bf = b_factor.astype(f32)
    cf = c_factor.astype(f32)
    sf = s_factor.astype(f32)
    osf = (f32(1.0) - sf).astype(f32)

    negbf = (-bf).astype(f32)
    nosf = (-osf).astype(f32)
    c2sat = (sf + osf * WS).astype(f32)
    cbv = ((f32(1.0) - cf) / f32(4096.0)).astype(f32)
    hc1 = (f32(1.0) - cf - (f32(1.0) - cf) * WS).astype(f32)
    hf6 = (f32(6.0) * h_factor.astype(f32)).astype(f32)

    o2 = np.zeros((P, P), dtype=f32)
    o2[:64, :64] = 1.0
    o2[64:, 64:] = 1.0

    rows = np.arange(64, dtype=f32)
    # hat weights per sample: w[b, in, out] = relu(1 - |pos[b, out] - in|)
    wy = np.maximum(f32(0.0), f32(1.0) - np.abs(ysv[:, None, :] - rows[None, :, None])).astype(f32)
    wx = np.maximum(f32(0.0), f32(1.0) - np.abs(xsv[:, None, :] - rows[None, :, None])).astype(f32)

    per_core = []
    n_cores = B // b_core
    for k in range(n_cores):
        sl = slice(k * b_core, (k + 1) * b_core)
        fac = np.zeros((ngroup, P, NFAC * G), dtype=f32)
        vals = np.stack([negbf[sl], cf[sl], sf[sl], nosf[sl],
                         c2sat[sl], cbv[sl], hc1[sl], hf6[sl]], -1)  # [b_core, 8]
        vals = vals.reshape(ngroup, G, 2, NFAC)
        for s, rws in ((0, slice(0, 64)), (1, slice(64, 128))):
            v = vals[:, :, s, :].reshape(ngroup, 1, G * NFAC)
            fac[:, rws, :] = np.broadcast_to(v, (ngroup, 64, G * NFAC))

        # block-diag image: [g, (s,row) 128, gg, c, (s', col) 128]
        xc = x[sl].astype(f32).reshape(ngroup, G, 2, 3, 64, 64)  # g, gg, s, c, row, col
        xblk = np.zeros((ngroup, 2, 64, G, 3, 2, 64), dtype=f32)  # g, s, row, gg, c, s', col
        xblk[:, 0, :, :, :, 0, :] = xc[:, :, 0].transpose(0, 3, 1, 2, 4)  # A rows, A cols
        xblk[:, 1, :, :, :, 1, :] = xc[:, :, 1].transpose(0, 3, 1, 2, 4)  # B rows, B cols
        xblk = xblk.reshape(ngroup, P, 384 * G)

        # ry weights: [g, (s,in-row) 128, gg, (s', out-row) 128], diag blocks only
        wyc = wy[sl].reshape(ngroup, G, 2, 64, 64)               # g, gg, s, in, out
        ryw = np.zeros((ngroup, 2, 64, G, 2, 64), dtype=f32)     # g, s, in, gg, s', out
        ryw[:, 0, :, :, 0, :] = wyc[:, :, 0].transpose(0, 2, 1, 3)
        ryw[:, 1, :, :, 1, :] = wyc[:, :, 1].transpose(0, 2, 1, 3)
        ryw = ryw.reshape(ngroup, P, 128 * G)

        # rx weights: [g, (s,col) 128, gg, out-col 64] (dense, sample by half)
        wxc = wx[sl].reshape(ngroup, G, 2, 64, 64)               # g, gg, s, col, out
        rxw = wxc.transpose(0, 2, 3, 1, 4).reshape(ngroup, P, OUT * G)

        imgry = np.concatenate([xblk, ryw], axis=2)
        rxfac = np.concatenate([rxw, fac], axis=2)
        per_core.append({
            "imgry": np.ascontiguousarray(imgry),
            "rxfac": np.ascontiguousarray(rxfac),
            "o2": o2,
        })
    return per_core


def unpermute(out_arr, b_core=B_CORE, gpairs=GPAIRS):
    """[ngroup, 128, 3*G*64] device layout -> [b_core, 3, 64, 64]."""
    ngroup = (b_core // 2) // gpairs
    o = out_arr.reshape(ngroup, 2, 64, gpairs, 3, 64)     # g, s, r, gg, c, col
    o = o.transpose(0, 3, 1, 4, 2, 5)                      # g, gg, s, c, r, col
    return np.ascontiguousarray(o.reshape(b_core, 3, 64, 64))


_NC_CACHE = {}


def kernel(**inputs):
    x = np.asarray(inputs["x"], dtype=np.float32)
    args = {k: np.asarray(inputs[k]) for k in
            ("flip_mask", "crop_i", "crop_j", "crop_h", "crop_w",
             "b_factor", "c_factor", "s_factor", "h_factor")}
    in_maps = host_prep(x, args["flip_mask"], args["crop_i"], args["crop_j"],
                        args["crop_h"], args["crop_w"], args["b_factor"],
                        args["c_factor"], args["s_factor"], args["h_factor"])
    key = (B_CORE, GPAIRS)
    if key not in _NC_CACHE:
        _NC_CACHE[key] = build_nc(B_CORE, GPAIRS)
    nc = _NC_CACHE[key]
    res = run_bass_kernel_spmd(nc, in_maps, list(range(N_CORES)))
    outs = [unpermute(np.asarray(r["out"])) for r in res.results]
    return np.concatenate(outs, axis=0).astype(np.float32)


if __name__ == "__main__":
    nc = build_nc()
    print("built ok")


# revision 6
# speedup vs baseline: 1.7839x; 1.0070x over previous
"""Trainium2 Bass kernel for nn_DataAugmentation (flip + resized-crop +
brightness/contrast/saturation/hue) — 8-core data-parallel, v2.

Engine-balanced rewrite of the baseline:
- hat interp weights built on DVE (1 fused op per tile) instead of 2 ACT passes
- brightness eviction and contrast on ACT via negated-relu trick (upper clamp
  free): n1 = relu(1 - bf*T2), n2 = relu(cf*n1 + bias8); downstream custom ops
  un-negate for free
- contrast mean via DVE tensor_reduce (1 op) instead of 16 ACT accumulates
- hue: cre-scaled trapezoid (no reciprocal), sector select via telescoping
  mask blend in 2 fused DVE ops, per-channel out in 1 fused op + Pool subtract
- Pool (GpSimd) engine offloads: channel diffs, max-chain, blends, final subs
- output written contiguously per group; host un-permutes
"""

import numpy as np

import concourse.bass as bass
import concourse.bacc as bacc
import concourse.tile as tile
import concourse.mybir as mybir
from concourse.bass_utils import run_bass_kernel_spmd
from concourse.dve_spec import (
    Spec, Src0, Src1, C0, C1, C2, Zero, One, maxx, minn, Bin, AluOp, lower,
)
from concourse.dve_ops import DveOp, DveOpSpec, OPS, CUSTOM_DVE_SPECS, _SUB_OPCODE_FOR_NAME, has_src1

F32 = mybir.dt.float32
P = 128
OUT = 64
N_CORES = 8
B_FULL = 4096
B_CORE = B_FULL // N_CORES          # 512
GPAIRS = 16                         # pairs per group
NPAIR = B_CORE // 2                 # 256
NGROUP = NPAIR // GPAIRS            # 16
NFAC = 8
W_R = np.float32(0.2989)
W_G = np.float32(0.587)
W_B = np.float32(0.114)
WS = np.float32(W_R + W_G + W_B)


# ---------------------------------------------------------------- custom ops
def _register_op(name, spec):
    if name in _SUB_OPCODE_FOR_NAME:
        for o in OPS:
            if o.name == name:
                return o
    opc = 1 + len(OPS)
    _SUB_OPCODE_FOR_NAME[name] = opc
    shas = {}
    for ver in ("v3", "v4"):
        try:
            s = DveOpSpec(name=name, opcode=opc, uops=lower(spec, ver=ver),
                          rd1_en=has_src1(spec))
            shas[ver] = s.sha(ver)
        except ValueError:
            pass
    op = DveOp(name, spec, subdim=False, uops_sha=shas)
    OPS.append(op)
    CUSTOM_DVE_SPECS[name] = spec
    return op


def _refbc(v, like):
    if isinstance(v, np.ndarray) and v.ndim >= 1:
        return v.reshape(v.shape[0], *([1] * (like.ndim - 1))).astype(np.float32)
    return np.float32(v)


def _refsame(v, like):
    if v.shape == like.shape:
        return v
    if v.size == like.size:
        return v.reshape(like.shape)
    return np.broadcast_to(v.reshape(v.shape[0], 1, -1) if v.ndim == 2 else v, like.shape)


def _absd(a, b):
    return Bin(AluOp.ABSOLUTE_DIFF, a, b)


def _cl01(x):
    return np.clip(x, 0.0, 1.0).astype(np.float32)


# hat(x) = relu(1 - |x - c0|): bilinear interp weight (c0 = k+2 per partition)
HAT = _register_op("AG2_HAT", Spec(
    body=maxx(One - _absd(Src0, C0), Zero),
    reference=lambda in0, in1, s0, s1, imm2:
        np.maximum(1.0 - np.abs(in0 - _refbc(s0, in0)), 0.0).astype(np.float32),
))
# g = c0*in0 + c1*in1
G2 = _register_op("AG2_G2", Spec(
    body=C0 * Src0 + C1 * Src1,
    reference=lambda in0, in1, s0, s1, imm2:
        (np.float32(s0) * in0 + np.float32(s1) * _refsame(in1, in0)).astype(np.float32),
))
# g = in1 + c0*in0
G2ACC = _register_op("AG2_G2ACC", Spec(
    body=Src1 + C0 * Src0,
    reference=lambda in0, in1, s0, s1, imm2:
        (_refsame(in1, in0) + np.float32(s0) * in0).astype(np.float32),
))
# g = c0*cl01(in0) + c1*cl01(in1)
G2N = _register_op("AG2_G2N", Spec(
    body=C0 * minn(maxx(Src0, Zero), One) + C1 * minn(maxx(Src1, Zero), One),
    reference=lambda in0, in1, s0, s1, imm2:
        (np.float32(s0) * _cl01(in0) + np.float32(s1) * _cl01(_refsame(in1, in0))).astype(np.float32),
))
# g = in1 + c0*cl01(in0)
G2NACC = _register_op("AG2_G2NACC", Spec(
    body=Src1 + C0 * minn(maxx(Src0, Zero), One),
    reference=lambda in0, in1, s0, s1, imm2:
        (_refsame(in1, in0) + np.float32(s0) * _cl01(in0)).astype(np.float32),
))
# x3 = cl01(gb - c0*cl01(n2));  in0 = n2, in1 = gb (broadcast)
SATCL = _register_op("AG2_SATCL", Spec(
    body=minn(maxx(Src1 - C0 * minn(maxx(Src0, Zero), One), Zero), One),
    reference=lambda in0, in1, s0, s1, imm2:
        _cl01(_refsame(in1, in0) - _refbc(s0, in0) * _cl01(in0)),
))
# cre = max(|e1|, |e2|, |e1+e2|)
CREH = _register_op("AG2_CREH", Spec(
    body=maxx(maxx(_absd(Src0, Zero), _absd(Src1, Zero)), _absd(Src0, Zero - Src1)),
    reference=lambda in0, in1, s0, s1, imm2: np.maximum(
        np.maximum(np.abs(in0), np.abs(_refsame(in1, in0))),
        np.abs(in0 + _refsame(in1, in0))).astype(np.float32),
))


def _m1(e1, e2):
    s = e1 + e2
    return (s > Zero) | (e2 > Zero)          # not r-max


def _m2(e1, e2):
    return (e2 > Zero) & (e1 < Zero)         # b strictly max


def _refm1(in0, in1):
    return ((in0 + in1) > 0) | (in1 > 0)


def _refm2(in0, in1):
    return (in1 > 0) & (in0 < 0)


# t = e1 + (e2-e1)*m1
OPA = _register_op("AG2_OPA", Spec(
    body=Src0 + (Src1 - Src0) * _m1(Src0, Src1),
    reference=lambda in0, in1, s0, s1, imm2: (
        in0 + (_refsame(in1, in0) - in0) * _refm1(in0, _refsame(in1, in0))
    ).astype(np.float32),
))
# u = (e3-e2)*m2 = (-e1-2*e2)*m2
OPB = _register_op("AG2_OPB", Spec(
    body=(Zero - ((Src0 + Src1) + Src1)) * _m2(Src0, Src1),
    reference=lambda in0, in1, s0, s1, imm2: (
        (-(in0 + 2.0 * _refsame(in1, in0))) * _refm2(in0, _refsame(in1, in0))
    ).astype(np.float32),
))
# zh = m1 + m2   (zb/2)
ZBH = _register_op("AG2_ZBH", Spec(
    body=_m1(Src0, Src1) + _m2(Src0, Src1),
    reference=lambda in0, in1, s0, s1, imm2: (
        _refm1(in0, _refsame(in1, in0)).astype(np.float32)
        + _refm2(in0, _refsame(in1, in0)).astype(np.float32)
    ).astype(np.float32),
))
# wz = (zh + zh + c0) * cre   (c0 = hf6 per pair)
ZWH = _register_op("AG2_ZWH", Spec(
    body=((Src0 + Src0) + C0) * Src1,
    reference=lambda in0, in1, s0, s1, imm2:
        (((in0 + in0) + _refbc(s0, in0)) * _refsame(in1, in0)).astype(np.float32),
))
# qw = clamp(2*cre - ||v2 - c0*cre| - c1*cre|, 0, cre); in0 = v2, in1 = cre
QTRIM = _register_op("AG2_QTRIM", Spec(
    body=minn(maxx(Src1 + Src1 - _absd(_absd(Src0, C0 * Src1), C1 * Src1), Zero), Src1),
    reference=lambda in0, in1, s0, s1, imm2: (lambda cre: np.minimum(np.maximum(
        2.0 * cre - np.abs(np.abs(in0 - np.float32(s0) * cre) - np.float32(s1) * cre),
        0.0), cre))(_refsame(in1, in0)).astype(np.float32),
))


# ---------------------------------------------------------------- device program
def build_nc(b_core=B_CORE, gpairs=GPAIRS, debug=False):
    npair = b_core // 2
    ngroup = npair // gpairs
    assert ngroup * gpairs == npair
    G = gpairs
    FDP = OUT * G          # per-pixel free size per group (1024)
    AluT = mybir.AluOpType
    Act = mybir.ActivationFunctionType

    nc = bacc.Bacc("TRN2", target_bir_lowering=False, debug=debug)

    imgry_in = nc.dram_tensor("imgry", [ngroup, P, 512 * G], F32, kind="ExternalInput")
    rxfac_in = nc.dram_tensor("rxfac", [ngroup, P, (OUT + NFAC) * G], F32, kind="ExternalInput")
    o2_in = nc.dram_tensor("o2", [P, P], F32, kind="ExternalInput")
    out_d = nc.dram_tensor("out", [ngroup, P, 3 * FDP], F32, kind="ExternalOutput")

    with tile.TileContext(nc) as tc:
        with tc.tile_pool(name="persist", bufs=1) as pers, \
             tc.tile_pool(name="grp", bufs=2) as grp, \
             tc.tile_pool(name="hue", bufs=2) as hue, \
             tc.tile_pool(name="pp", bufs=2, space="PSUM") as pp:

            O2 = pers.tile([P, P], F32)
            nc.sync.dma_start(O2[:], o2_in[:])

            imgryd = [pers.tile([P, 512 * G], F32, tag=f"imgry{i}", name=f"imgry{i}") for i in range(3)]

            def front(g, p0, gs, ui):
                p1 = p0 + gs
                imgry = imgryd[ui % 3]
                imgr = imgry[:, 0:384 * G].rearrange("p (gg c k) -> p gg c k", c=3, k=128)
                ryr = imgry[:, 384 * G:512 * G].rearrange("p (gg k) -> p gg k", k=128)

                # ---- input DMAs: host-prepacked block-diag img + hat weights
                if gs == G:
                    nc.sync.dma_start(imgry[:], imgry_in[g])
                else:
                    nc.sync.dma_start(imgry[:, 384 * p0:384 * p1],
                                      imgry_in[g][:, 384 * p0:384 * p1])
                    nc.sync.dma_start(imgry[:, 384 * G + 128 * p0:384 * G + 128 * p1],
                                      imgry_in[g][:, 384 * G + 128 * p0:384 * G + 128 * p1])
                rxfac = grp.tile([P, (OUT + NFAC) * G], F32, tag="rxfac", bufs=3)
                rxr = rxfac[:, 0:OUT * G].rearrange("p (gg k) -> p gg k", k=OUT)
                if gs == G:
                    nc.sync.dma_start(rxfac[:], rxfac_in[g])
                else:
                    nc.sync.dma_start(rxfac[:, OUT * p0:OUT * p1],
                                      rxfac_in[g][:, OUT * p0:OUT * p1])
                    nc.sync.dma_start(rxfac[:, OUT * G + NFAC * p0:OUT * G + NFAC * p1],
                                      rxfac_in[g][:, OUT * G + NFAC * p0:OUT * G + NFAC * p1])
                facr = rxfac[:, OUT * G:].rearrange("p (gg s) -> p gg s", s=NFAC)

                # ---- resize matmuls + eviction
                n1 = grp.tile([P, 192 * G], F32, tag="n1", bufs=3)
                n1r = n1[:].rearrange("p (gg c k) -> p gg c k", c=3, k=OUT)

                def stage2(b, sbt):
                    for half in range(2):
                        p = 2 * b + half
                        T2 = pp.tile([P, 192], F32, tag="t2", bufs=2)
                        for c in range(3):
                            nc.tensor.matmul(
                                T2[:, 64 * c:64 * (c + 1)],
                                sbt[:, 384 * half + 128 * c:384 * half + 128 * (c + 1)],
                                rxr[:, p, :], start=True, stop=True)
                        # n1 = relu(1 - bf*T2)   (negated brightness, clamped)
                        nc.scalar.activation(
                            n1[:, 192 * p:192 * (p + 1)], T2[:], Act.Relu,
                            bias=1.0, scale=facr[:, p, 0:1])

                prev = None
                for b in range(p0 // 2, p1 // 2):
                    T1D = pp.tile([P, 768], F32, tag="t1", bufs=2)
                    for half in range(2):
                        p = 2 * b + half
                        for c in range(3):
                            nc.tensor.matmul(
                                T1D[:, 384 * half + 128 * c:384 * half + 128 * (c + 1)],
                                imgr[:, p, c, :], ryr[:, p, :],
                                start=True, stop=True)
                    sbt = grp.tile([P, 768], F32, tag="sbt", bufs=3)
                    nc.scalar.copy(sbt[:], T1D[:])
                    if prev is not None:
                        stage2(*prev)
                    prev = (b, sbt)
                stage2(*prev)

                # ---- contrast mean path: g1n = sum w_c * n1_c ; tb from sums
                g1n = grp.tile([P, FDP], F32, tag="gsum")
                g1r = g1n[:].rearrange("p (gg k) -> p gg k", k=OUT)
                nc.vector._custom_dve(G2, out=g1r[:, p0:p1, :], in0=n1r[:, p0:p1, 0, :],
                                      in1=n1r[:, p0:p1, 1, :], s0=float(W_R), s1=float(W_G))
                nc.vector._custom_dve(G2ACC, out=g1r[:, p0:p1, :], in0=n1r[:, p0:p1, 2, :],
                                      in1=g1r[:, p0:p1, :], s0=float(W_B))
                mrow = grp.tile([P, G], F32, tag="mrow")
                nc.vector.tensor_reduce(mrow[:, p0:p1], g1r[:, p0:p1, :],
                                        mybir.AxisListType.X, AluT.add)
                Mcol = pp.tile([P, G], F32, tag="mcol", bufs=1)
                nc.tensor.matmul(Mcol[:, p0:p1], O2[:], mrow[:, p0:p1], start=True, stop=True)
                bias8 = grp.tile([P, G], F32, tag="bias8")
                nc.vector.tensor_tensor(bias8[:, p0:p1], Mcol[:, p0:p1],
                                        facr[:, p0:p1, 5], AluT.mult)
                nc.vector.tensor_tensor(bias8[:, p0:p1], bias8[:, p0:p1],
                                        facr[:, p0:p1, 6], AluT.add)

                return dict(g=g, p0=p0, p1=p1, facr=facr, n1=n1, n1r=n1r,
                            g1n=g1n, g1r=g1r, bias8=bias8)

            def mid(st):
                g = st["g"]
                p0 = st["p0"]
                p1 = st["p1"]
                facr = st["facr"]
                n1 = st["n1"]
                n1r = st["n1r"]
                g1n = st["g1n"]
                g1r = st["g1r"]
                bias8 = st["bias8"]

                # ---- contrast: n2 = relu(cf*n1 + bias8) in place
                for p in range(p0, p1):
                    nc.scalar.activation(
                        n1[:, 192 * p:192 * (p + 1)], n1[:, 192 * p:192 * (p + 1)],
                        Act.Relu, bias=bias8[:, p:p + 1], scale=facr[:, p, 1:2])

                # ---- saturation: g2n, gb (Pool), x3 (in place on n1/n2)
                nc.vector._custom_dve(G2N, out=g1r[:, p0:p1, :], in0=n1r[:, p0:p1, 0, :],
                                      in1=n1r[:, p0:p1, 1, :], s0=float(W_R), s1=float(W_G))
                nc.vector._custom_dve(G2NACC, out=g1r[:, p0:p1, :], in0=n1r[:, p0:p1, 2, :],
                                      in1=g1r[:, p0:p1, :], s0=float(W_B))
                gb = hue.tile([P, FDP], F32, tag="gb", bufs=2)
                for p in range(p0, p1):
                    nc.scalar.activation(
                        gb[:, OUT * p:OUT * (p + 1)], g1n[:, OUT * p:OUT * (p + 1)],
                        Act.Identity, bias=facr[:, p, 4:5], scale=facr[:, p, 3:4])
                for p in range(p0, p1):
                    nc.vector._custom_dve(
                        SATCL, out=n1r[:, p, :, :], in0=n1r[:, p, :, :],
                        in1=gb[:, OUT * p:OUT * (p + 1)].unsqueeze(1).broadcast_to((P, 3, OUT)),
                        s0=facr[:, p, 2:3])

            def tail(st):
                g = st["g"]
                p0 = st["p0"]
                p1 = st["p1"]
                fl0 = OUT * p0
                fl1 = OUT * p1
                facr = st["facr"]
                n1 = st["n1"]
                n1r = st["n1r"]
                x3r = n1r

                # ---- hue
                r_s = x3r[:, p0:p1, 0, :]
                g_s = x3r[:, p0:p1, 1, :]
                b_s = x3r[:, p0:p1, 2, :]
                e1 = hue.tile([P, FDP], F32, tag="e1", bufs=1)
                e2 = hue.tile([P, FDP], F32, tag="e2", bufs=1)
                cre = hue.tile([P, FDP], F32, tag="cre", bufs=1)
                ta = hue.tile([P, FDP], F32, tag="ta", bufs=1)
                ub = hue.tile([P, FDP], F32, tag="ub", bufs=1)
                zh = hue.tile([P, FDP], F32, tag="zh", bufs=1)
                hfc = hue.tile([P, FDP], F32, tag="hfc", bufs=1)
                v2 = hue.tile([P, FDP], F32, tag="v2", bufs=1)
                mxt = hue.tile([P, FDP], F32, tag="e2", bufs=1)
                e1r = e1[:].rearrange("p (gg k) -> p gg k", k=OUT)
                e2r = e2[:].rearrange("p (gg k) -> p gg k", k=OUT)
                mxr = mxt[:].rearrange("p (gg k) -> p gg k", k=OUT)

                nc.vector.tensor_tensor(e1r[:, p0:p1, :], g_s, b_s, AluT.subtract)
                nc.vector.tensor_tensor(e2r[:, p0:p1, :], b_s, r_s, AluT.subtract)
                nc.vector._custom_dve(CREH, out=cre[:, fl0:fl1], in0=e1[:, fl0:fl1], in1=e2[:, fl0:fl1])
                nc.vector._custom_dve(OPA, out=ta[:, fl0:fl1], in0=e1[:, fl0:fl1], in1=e2[:, fl0:fl1])
                nc.vector._custom_dve(OPB, out=ub[:, fl0:fl1], in0=e1[:, fl0:fl1], in1=e2[:, fl0:fl1])
                nc.vector._custom_dve(ZBH, out=zh[:, fl0:fl1], in0=e1[:, fl0:fl1], in1=e2[:, fl0:fl1])
                nc.vector.tensor_tensor(ta[:, fl0:fl1], ta[:, fl0:fl1], ub[:, fl0:fl1], AluT.add)   # esel
                # wz = (2*zh + hf6) * cre (per pair)
                for p in range(p0, p1):
                    nc.vector._custom_dve(
                        ZWH, out=hfc[:, OUT * p:OUT * (p + 1)],
                        in0=zh[:, OUT * p:OUT * (p + 1)],
                        in1=cre[:, OUT * p:OUT * (p + 1)],
                        s0=facr[:, p, 7:8])
                nc.vector.tensor_tensor(v2[:, fl0:fl1], ta[:, fl0:fl1], hfc[:, fl0:fl1], AluT.add)
                nc.vector.tensor_tensor(mxr[:, p0:p1, :], r_s, g_s, AluT.max)
                nc.vector.tensor_tensor(mxr[:, p0:p1, :], mxr[:, p0:p1, :], b_s, AluT.max)
                for ci, cen in enumerate((0.0, 2.0, 4.0)):
                    qt = hue.tile([P, FDP], F32, tag="qt", bufs=1)
                    nc.vector._custom_dve(QTRIM, out=qt[:, fl0:fl1], in0=v2[:, fl0:fl1],
                                          in1=cre[:, fl0:fl1], s0=cen, s1=3.0)
                    nc.vector.tensor_tensor(x3r[:, p0:p1, ci, :],
                                            mxr[:, p0:p1, :],
                                            qt[:, fl0:fl1].rearrange("p (gg k) -> p gg k", k=OUT),
                                            AluT.subtract)

                nc.gpsimd.dma_start(out_d[g][:, 192 * p0:192 * p1],
                                    n1[:, 192 * p0:192 * p1])

            units = ([(0, 0, 2), (0, 2, 2), (0, 4, 4), (0, 8, 8)]
                     + [(1, 0, 8), (1, 8, 8)]
                     + [(g, 0, G) for g in range(2, ngroup - 1)]
                     + [(ngroup - 1, 0, 8), (ngroup - 1, 8, 4),
                        (ngroup - 1, 12, 2), (ngroup - 1, 14, 2)])
            nu = len(units)
            sts = {}
            for i, (g, p0, gs) in enumerate(units):
                if i >= 2:
                    tail(sts.pop(i - 2))
                if i >= 1:
                    mid(sts[i - 1])
                sts[i] = front(g, p0, gs, i)
            mid(sts[nu - 1])
            tail(sts.pop(nu - 2))
            tail(sts.pop(nu - 1))

    nc.compile()
    return nc


# ---------------------------------------------------------------- host prep
def host_prep(x, flip_mask, crop_i, crop_j, crop_h, crop_w,
              b_factor, c_factor, s_factor, h_factor,
              b_core=B_CORE, gpairs=GPAIRS):
    f32 = np.float32
    B = x.shape[0]
    npair = b_core // 2
    ngroup = npair // gpairs
    G = gpairs

    ar = (np.arange(OUT, dtype=f32) + f32(0.5))
    ys = crop_i[:, None].astype(f32) + ar[None, :] * (crop_h.astype(f32)[:, None] / f32(OUT)) - f32(0.5)
    xs = crop_j[:, None].astype(f32) + ar[None, :] * (crop_w.astype(f32)[:, None] / f32(OUT)) - f32(0.5)

    def eff(p):
        return np.where(p < 0, p + f32(1.0), np.minimum(p, f32(63.0))).astype(f32)

    ysv = eff(ys)
    xsv = np.where(flip_mask[:, None], f32(63.0) - eff(xs), eff(xs)).astype(f32)

    bf = b_factor.astype(f32)
    cf = c_factor.astype(f32)
    sf = s_factor.astype(f32)
    osf = (f32(1.0) - sf).astype(f32)

    negbf = (-bf).astype(f32)
    nosf = (-osf).astype(f32)
    c2sat = (sf + osf * WS).astype(f32)
    cbv = ((f32(1.0) - cf) / f32(4096.0)).astype(f32)
    hc1 = (f32(1.0) - cf - (f32(1.0) - cf) * WS).astype(f32)
    hf6 = (f32(6.0) * h_factor.astype(f32)).astype(f32)

    o2 = np.zeros((P, P), dtype=f32)
    o2[:64, :64] = 1.0
    o2[64:, 64:] = 1.0

    rows = np.arange(64, dtype=f32)
    # hat weights per sample: w[b, in, out] = relu(1 - |pos[b, out] - in|)
    wy = np.maximum(f32(0.0), f32(1.0) - np.abs(ysv[:, None, :] - rows[None, :, None])).astype(f32)
    wx = np.maximum(f32(0.0), f32(1.0) - np.abs(xsv[:, None, :] - rows[None, :, None])).astype(f32)

    per_core = []
    n_cores = B // b_core
    for k in range(n_cores):
        sl = slice(k * b_core, (k + 1) * b_core)
        fac = np.zeros((ngroup, P, NFAC * G), dtype=f32)
        vals = np.stack([negbf[sl], cf[sl], sf[sl], nosf[sl],
                         c2sat[sl], cbv[sl], hc1[sl], hf6[sl]], -1)  # [b_core, 8]
        vals = vals.reshape(ngroup, G, 2, NFAC)
        for s, rws in ((0, slice(0, 64)), (1, slice(64, 128))):
            v = vals[:, :, s, :].reshape(ngroup, 1, G * NFAC)
            fac[:, rws, :] = np.broadcast_to(v, (ngroup, 64, G * NFAC))

        # block-diag image: [g, (s,row) 128, gg, c, (s', col) 128]
        xc = x[sl].astype(f32).reshape(ngroup, G, 2, 3, 64, 64)  # g, gg, s, c, row, col
        xblk = np.zeros((ngroup, 2, 64, G, 3, 2, 64), dtype=f32)  # g, s, row, gg, c, s', col
        xblk[:, 0, :, :, :, 0, :] = xc[:, :, 0].transpose(0, 3, 1, 2, 4)  # A rows, A cols
        xblk[:, 1, :, :, :, 1, :] = xc[:, :, 1].transpose(0, 3, 1, 2, 4)  # B rows, B cols
        xblk = xblk.reshape(ngroup, P, 384 * G)

        # ry weights: [g, (s,in-row) 128, gg, (s', out-row) 128], diag blocks only
        wyc = wy[sl].reshape(ngroup, G, 2, 64, 64)               # g, gg, s, in, out
        ryw = np.zeros((ngroup, 2, 64, G, 2, 64), dtype=f32)     # g, s, in, gg, s', out
        ryw[:, 0, :, :, 0, :] = wyc[:, :, 0].transpose(0, 2, 1, 3)
        ryw[:, 1, :, :, 1, :] = wyc[:, :, 1].transpose(0, 2, 1, 3)
        ryw = ryw.reshape(ngroup, P, 128 * G)

        # rx weights: [g, (s,col) 128, gg, out-col 64] (dense, sample by half)
        wxc = wx[sl].reshape(ngroup, G, 2, 64, 64)               # g, gg, s, col, out
        rxw = wxc.transpose(0, 2, 3, 1, 4).reshape(ngroup, P, OUT * G)

        imgry = np.concatenate([xblk, ryw], axis=2)
        rxfac = np.concatenate([rxw, fac], axis=2)
        per_core.append({
            "imgry": np.ascontiguousarray(imgry),
            "rxfac": np.ascontiguousarray(rxfac),
            "o2": o2,
        })
    return per_core


def unpermute(out_arr, b_core=B_CORE, gpairs=GPAIRS):
    """[ngroup, 128, 3*G*64] device layout -> [b_core, 3, 64, 64]."""
    ngroup = (b_core // 2) // gpairs
    o = out_arr.reshape(ngroup, 2, 64, gpairs, 3, 64)     # g, s, r, gg, c, col
    o = o.transpose(0, 3, 1, 4, 2, 5)                      # g, gg, s, c, r, col
    return np.ascontiguousarray(o.reshape(b_core, 3, 64, 64))


_NC_CACHE = {}


def kernel(**inputs):
    x = np.asarray(inputs["x"], dtype=np.float32)
    args = {k: np.asarray(inputs[k]) for k in
            ("flip_mask", "crop_i", "crop_j", "crop_h", "crop_w",
             "b_factor", "c_factor", "s_factor", "h_factor")}
    in_maps = host_prep(x, args["flip_mask"], args["crop_i"], args["crop_j"],
                        args["crop_h"], args["crop_w"], args["b_factor"],
                        args["c_factor"], args["s_factor"], args["h_factor"])
    key = (B_CORE, GPAIRS)
    if key not in _NC_CACHE:
        _NC_CACHE[key] = build_nc(B_CORE, GPAIRS)
    nc = _NC_CACHE[key]
    res = run_bass_kernel_spmd(nc, in_maps, list(range(N_CORES)))
    outs = [unpermute(np.asarray(r["out"])) for r in res.results]
    return np.concatenate(outs, axis=0).astype(np.float32)


if __name__ == "__main__":
    nc = build_nc()
    print("built ok")
